# revision 2
# baseline (speedup 1.0000x reference)
"""Cross-attention kernel for Trainium2 (8 NeuronCores, Bass/Tile).

Sharding: core c handles batch b = c//2 and head-group hg = c%2 (8 of 16
heads).  Each core computes, for its (b, hg):
  - q/k/v projections in f16 (weights column-sliced per head group)
  - per-head masked softmax attention (scores in PSUM f32, mask folded in
    via an identity-matmul accumulation, exp+row-sums on the scalar engine)
  - att-mean partial in f16 (weighted head sum on the vector engine)
  - y partial = att @ v @ Wp[:, hg-cols].T in f16 -> host adds halves + bias
Engine budget per 128-row t-chunk (16 chunks): PE ~8.5us (QK+mask+AV+proj),
ACT ~9us (exp+Z), DVE ~7us (att-mean f16, y-normalize, PSUM->SBUF copies),
sync ~9.7us (prob transpose via xbar DMA), gpsimd (SWDGE DMA triggers).
"""

import os
import sys

sys.path.insert(0, "/opt/trn_rl_repo")

import numpy as np

import concourse.bass as bass
import concourse.tile as tile
from concourse import mybir
import concourse.bass_utils as bu

# ---------------------------------------------------------------- constants
B, T, TE, C = 4, 2048, 1024, 1024
H = 16          # total heads
HG = 8          # heads per group (per core)
D = 64          # head dim
KT = 8          # contraction tiles for Q/K (bias added in PSUM->SBUF copy)
KTV = 9         # V keeps a ones-row for its bias (v is s-major, bias on free dim)
CV_PAD = KTV * 128
EXPB = -2.0     # constant exp bias (cancels in softmax, guards fp16 overflow)
NEG = -30000.0  # additive mask value (exp underflows to exactly 0)
N_CORES = 8

f32 = mybir.dt.float32
f16 = mybir.dt.float16


def _split_waits(nc, max_waits=1):
    """walrus in this container accepts at most one sync-wait command per
    instruction; hoist extra waits onto preceding same-engine NoOps."""
    import bass_rust

    ctr = 0
    for f in nc.m.functions:
        for blk in f.blocks:
            il = list(blk.instructions)
            out = []
            changed = False
            for inst in il:
                si = inst.sync_info
                if si is not None and si.on_wait and len(si.on_wait) > max_waits:
                    waits = list(si.on_wait)
                    for w in waits[:-max_waits]:
                        ctr += 1
                        nop = mybir.InstNoOp(name=f"waitsplit_{ctr}", ins=[], outs=[])
                        nop.engine = inst.engine
                        nop.sync_info = bass_rust.SyncInfo(on_wait=[w], on_update=[])
                        out.append(nop)
                    inst.sync_info = bass_rust.SyncInfo(
                        on_wait=waits[-max_waits:],
                        on_update=list(si.on_update) if si.on_update else [],
                    )
                    changed = True
                out.append(inst)
            if changed:
                blk.instructions = out


def _build_program():
    nc = bass.Bass("TRN2", target_bir_lowering=False, debug=False)

    xta_d = nc.declare_dram_parameter("xta", [C, T], f16, isOutput=False)
    eta_d = nc.declare_dram_parameter("eta", [CV_PAD, TE], f16, isOutput=False)
    wq_d = nc.declare_dram_parameter("wqta", [C, 512], f16, isOutput=False)
    wk_d = nc.declare_dram_parameter("wkta", [C, 512], f16, isOutput=False)
    wv_d = nc.declare_dram_parameter("wvta", [CV_PAD, 512], f16, isOutput=False)
    wp_d = nc.declare_dram_parameter("wpt", [512, C], f16, isOutput=False)
    bqk_d = nc.declare_dram_parameter("bqk", [128, 8], f32, isOutput=False)
    mneg_d = nc.declare_dram_parameter("mneg", [T, TE], f16, isOutput=False)
    idh_d = nc.declare_dram_parameter("identh", [128, 128], f16, isOutput=False)
    y_d = nc.declare_dram_parameter("ypart", [T, C], f16, isOutput=True)
    a_d = nc.declare_dram_parameter("apart", [T, TE], f16, isOutput=True)

    xta_r = xta_d.rearrange("(kt p) n -> p kt n", p=128)
    eta_r = eta_d.rearrange("(kt p) n -> p kt n", p=128)
    wq_r = wq_d.rearrange("(kt p) n -> p kt n", p=128)
    wk_r = wk_d.rearrange("(kt p) n -> p kt n", p=128)
    wv_r = wv_d.rearrange("(kt p) n -> p kt n", p=128)
    wp_r = wp_d.rearrange("(kt p) n -> p kt n", p=128)

    with tile.TileContext(nc) as tc:
        with tc.tile_pool(name="persist", bufs=1) as persist:
            qT = persist.tile([128, 4, T], f16, tag="qT")
            kT = persist.tile([128, 4, TE], f16, tag="kT")
            vsb = persist.tile([128, 8, 512], f16, tag="vsb")
            wp = persist.tile([128, 4, C], f16, tag="wp")
            idh = persist.tile([128, 128], f16, tag="idh")
            bqk = persist.tile([128, 8], f32, tag="bqk")
            eb = persist.tile([128, 1], f32, tag="eb")

            nc.scalar.dma_start(out=idh, in_=idh_d[:, :])
            nc.scalar.dma_start(out=bqk, in_=bqk_d[:, :])
            nc.vector.memset(eb, EXPB)
            nc.scalar.dma_start(out=wp, in_=wp_r[:, :, :])

            # ---------------- stage A: projections ----------------
            psA_ctx = tc.tile_pool(name="psA", bufs=2, space="PSUM")
            psA = psA_ctx.__enter__()
            with tc.tile_pool(name="wqpool", bufs=1) as wqpool:
                wq = wqpool.tile([128, KT, 512], f16, tag="wq")
                nc.scalar.dma_start(out=wq, in_=wq_r[:, :, :])
                with tc.tile_pool(name="xpool", bufs=2) as xpool:
                    for Tc in range(4):
                        tsl = slice(Tc * 512, (Tc + 1) * 512)
                        xt = xpool.tile([128, KT, 512], f16, tag="xt")
                        nc.sync.dma_start(out=xt, in_=xta_r[:, :, tsl])
                        for pt in range(4):
                            ps = psA.tile([128, 512], f32, tag="psA")
                            for kt in range(KT):
                                nc.tensor.matmul(
                                    ps[:, :],
                                    wq[:, kt, pt * 128:(pt + 1) * 128],
                                    xt[:, kt, :],
                                    start=(kt == 0), stop=(kt == KT - 1),
                                )
                            nc.scalar.activation(
                                qT[:, pt, tsl], ps[:, :],
                                mybir.ActivationFunctionType.Identity,
                                bias=bqk[:, pt:pt + 1],
                            )

            with tc.tile_pool(name="wkvpool", bufs=1) as wkvpool:
                wk = wkvpool.tile([128, KT, 512], f16, tag="wk")
                wv = wkvpool.tile([128, KTV, 512], f16, tag="wv")
                nc.scalar.dma_start(out=wk, in_=wk_r[:, :, :])
                nc.scalar.dma_start(out=wv, in_=wv_r[:, :, :])
                with tc.tile_pool(name="epool", bufs=2) as epool:
                    for sh in range(2):
                        ssl = slice(sh * 512, (sh + 1) * 512)
                        et = epool.tile([128, KTV, 512], f16, tag="et")
                        nc.sync.dma_start(out=et, in_=eta_r[:, :, ssl])
                        for pt in range(4):
                            ps = psA.tile([128, 512], f32, tag="psA")
                            for kt in range(KT):
                                nc.tensor.matmul(
                                    ps[:, :],
                                    wk[:, kt, pt * 128:(pt + 1) * 128],
                                    et[:, kt, :],
                                    start=(kt == 0), stop=(kt == KT - 1),
                                )
                            nc.scalar.activation(
                                kT[:, pt, ssl], ps[:, :],
                                mybir.ActivationFunctionType.Identity,
                                bias=bqk[:, 4 + pt:5 + pt],
                            )
                        for st4 in range(4):
                            ps = psA.tile([128, 512], f32, tag="psA")
                            for kt in range(KTV):
                                nc.tensor.matmul(
                                    ps[:, :],
                                    et[:, kt, st4 * 128:(st4 + 1) * 128],
                                    wv[:, kt, :],
                                    start=(kt == 0), stop=(kt == KTV - 1),
                                )
                            nc.scalar.copy(vsb[:, sh * 4 + st4, :], ps[:, :])
            psA_ctx.__exit__(None, None, None)

            # ---------------- stage B: attention ----------------
            with (
                tc.tile_pool(name="spool", bufs=2, space="PSUM") as spool,
                tc.tile_pool(name="ypool", bufs=2, space="PSUM") as ypool,
                tc.tile_pool(name="ppool", bufs=2, space="PSUM") as ppool,
                tc.tile_pool(name="attEpool", bufs=3) as attEpool,
                tc.tile_pool(name="attTpool", bufs=3) as attTpool,
                tc.tile_pool(name="mpool", bufs=2) as mpool,
                tc.tile_pool(name="zpool", bufs=2) as zpool,
                tc.tile_pool(name="rpool", bufs=2) as rpool,
                tc.tile_pool(name="accpool", bufs=2) as accpool,
                tc.tile_pool(name="ytpool", bufs=2) as ytpool,
                tc.tile_pool(name="bpool", bufs=2) as bpool,
                tc.tile_pool(name="opool", bufs=2) as opool,
                tc.tile_pool(name="drpool", bufs=1, space="DRAM") as drpool,
            ):
                rT_dram = drpool.tile([HG, T], f16, tag="rT")

                def av_proj(tci, aT, bc):
                    tsl = slice(tci * 128, (tci + 1) * 128)
                    yts = ytpool.tile([128, 4, 128], f16, tag="yts")
                    for hp in range(4):
                        yps = ypool.tile([128, 128], f32, tag="yps")
                        for st in range(8):
                            for h2 in range(2):
                                h = hp * 2 + h2
                                nc.tensor.matmul(
                                    yps[h2 * 64:(h2 + 1) * 64, :],
                                    vsb[:, st, h * 64:(h + 1) * 64],
                                    aT[:, h * 8 + st, :],
                                    start=(st == 0), stop=(st == 7),
                                    tile_position=(0, h2 * 64),
                                )
                        for h2 in range(2):
                            h = hp * 2 + h2
                            nc.vector.tensor_mul(
                                yts[h2 * 64:(h2 + 1) * 64, hp, :],
                                yps[h2 * 64:(h2 + 1) * 64, :],
                                bc[:, h, :],
                            )
                    # output projection (partial over this head group's columns)
                    for ch in range(2):
                        csl = slice(ch * 512, (ch + 1) * 512)
                        pps = ppool.tile([128, 512], f32, tag="pps")
                        for kt in range(4):
                            nc.tensor.matmul(
                                pps[:, :], yts[:, kt, :], wp[:, kt, csl],
                                start=(kt == 0), stop=(kt == 3),
                            )
                        oc = opool.tile([128, 512], f16, tag="oc")
                        nc.vector.tensor_copy(oc[:, :], pps[:, :])
                        nc.gpsimd.dma_start(out=y_d[tsl, csl], in_=oc[:, :])

                pending = None
                for tci in range(16):
                    tsl = slice(tci * 128, (tci + 1) * 128)
                    mk = mpool.tile([128, TE], f16, tag="mk")
                    nc.gpsimd.dma_start(out=mk, in_=mneg_d[tsl, :])
                    aE = attEpool.tile([128, HG, TE], f16, tag="aE")
                    aT = attTpool.tile([128, HG * 8, 128], f16, tag="aT")
                    Zs = zpool.tile([128, HG], f32, tag="Zs")
                    for hp in range(4):
                        S0 = spool.tile([128, TE], f32, tag="S")
                        S1 = spool.tile([128, TE], f32, tag="S")
                        for sh in range(2):
                            ssl = slice(sh * 512, (sh + 1) * 512)
                            for h2, S in ((0, S0), (1, S1)):
                                hrow = slice(h2 * 64, (h2 + 1) * 64)
                                nc.tensor.matmul(
                                    S[:, ssl],
                                    qT[hrow, hp, tsl],
                                    kT[hrow, hp, ssl],
                                    start=True, stop=False,
                                    tile_position=(h2 * 64, 0),
                                )
                        for h2, S in ((0, S0), (1, S1)):
                            for sh in range(2):
                                ssl = slice(sh * 512, (sh + 1) * 512)
                                nc.tensor.matmul(
                                    S[:, ssl], idh[:, :], mk[:, ssl],
                                    start=False, stop=True,
                                )
                        for h2, S in ((0, S0), (1, S1)):
                            h = hp * 2 + h2
                            nc.scalar.activation(
                                aE[:, h, :], S[:, :],
                                mybir.ActivationFunctionType.Exp,
                                bias=eb[:, 0:1],
                                accum_out=Zs[:, h:h + 1],
                            )
                    rc = zpool.tile([128, HG], f32, tag="rc")
                    nc.vector.reciprocal(rc[:, :], Zs[:, :])
                    # 1/Z -> f16, xbar-transpose [128,128] -> [h, t], broadcast
                    # to 64 partitions via a DRAM bounce for the AV normalize
                    rcp = rpool.tile([128, 128], f16, tag="rcp")
                    nc.vector.tensor_copy(rcp[:, 0:HG], rc[:, :])
                    rt = rpool.tile([128, 128], f16, tag="rt")
                    nc.sync.dma_start_transpose(rt, rcp)
                    nc.gpsimd.dma_start(out=rT_dram[:, tsl], in_=rt[0:HG, :])
                    bc = bpool.tile([64, HG, 128], f16, tag="bc")
                    nc.gpsimd.dma_start(
                        out=bc, in_=rT_dram[:, tsl].partition_broadcast(64)
                    )
                    # transpose attention probs for the AV contraction
                    nc.sync.dma_start_transpose(aT[:, :, :], aE[:, :, :])
                    # software pipeline: AV+proj run one chunk behind so the
                    # PE's in-order stream never stalls on exp/transpose
                    if pending is not None:
                        av_proj(*pending)
                    # att-mean partial (sum over this core's heads of att/Z;
                    # host applies the 1/16) -- issued after av_proj so the
                    # DVE drains the previous chunk's normalize/copies first
                    acc = accpool.tile([128, TE], f16, tag="acc")
                    nc.vector.tensor_scalar_mul(acc[:, :], aE[:, 0, :], rc[:, 0:1])
                    for h in range(1, HG):
                        nc.vector.scalar_tensor_tensor(
                            out=acc[:, :], in0=aE[:, h, :], scalar=rc[:, h:h + 1],
                            in1=acc[:, :],
                            op0=mybir.AluOpType.mult, op1=mybir.AluOpType.add,
                        )
                    nc.gpsimd.dma_start(out=a_d[tsl, :], in_=acc[:, :])
                    pending = (tci, aT, bc)
                av_proj(*pending)

    _split_waits(nc)
    return nc


_PROGRAM = None


def _get_program():
    global _PROGRAM
    if _PROGRAM is None:
        _PROGRAM = _build_program()
    return _PROGRAM


def _host_inputs(x, encoder_output, mask, Wq, bq, Wk, bk, Wv, bv, Wp, bp):
    """Build the 8 per-core input maps."""
    x = np.asarray(x, np.float32)
    enc = np.asarray(encoder_output, np.float32)
    mask = np.asarray(mask)
    scale = 1.0 / np.sqrt(D)
    identh = np.eye(128, dtype=np.float16)

    in_maps = []
    for c in range(N_CORES):
        b, hg = c // 2, c % 2
        hsl = slice(hg * 512, (hg + 1) * 512)

        xta = np.ascontiguousarray(x[b].T, dtype=np.float16)
        eta = np.zeros((CV_PAD, TE), np.float16)
        eta[:C] = enc[b].T
        eta[C] = 1.0

        wqta = np.ascontiguousarray(
            (np.asarray(Wq, np.float32)[hsl] * scale).T, dtype=np.float16)
        wkta = np.ascontiguousarray(
            np.asarray(Wk, np.float32)[hsl].T, dtype=np.float16)
        wvta = np.zeros((CV_PAD, 512), np.float16)
        wvta[:C] = np.asarray(Wv, np.float32)[hsl].T
        wvta[C] = np.asarray(bv, np.float32)[hsl]
        wpt = np.ascontiguousarray(
            np.asarray(Wp, np.float32)[:, hsl].T, dtype=np.float16)

        bqk = np.empty((128, 8), np.float32)
        bqk[:, 0:4] = (np.asarray(bq, np.float32)[hsl] * scale).reshape(4, 128).T
        bqk[:, 4:8] = np.asarray(bk, np.float32)[hsl].reshape(4, 128).T

        mneg = (mask[b].astype(np.float16)) * np.float16(NEG)

        in_maps.append({
            "xta": xta,
            "eta": eta,
            "wqta": wqta,
            "wkta": wkta,
            "wvta": wvta,
            "wpt": wpt,
            "bqk": bqk,
            "mneg": mneg,
            "identh": identh,
        })
    return in_maps


def kernel(x, encoder_output, mask, Wq, bq, Wk, bk, Wv, bv, Wp, bp):
    nc = _get_program()
    in_maps = _host_inputs(x, encoder_output, mask, Wq, bq, Wk, bk, Wv, bv, Wp, bp)
    trace = bool(int(os.environ.get("KERNEL_TRACE", "0")))
    res = bu.run_bass_kernel_spmd(nc, in_maps, list(range(N_CORES)), trace=trace)
    if trace:
        kernel.last_exec_time_ns = res.exec_time_ns
        kernel.last_profile = res
    outs = res.results

    bp = np.asarray(bp, np.float32)
    y = np.empty((B, T, C), np.float32)
    am = np.empty((B, T, TE), np.float32)
    for b in range(B):
        y[b] = (outs[2 * b]["ypart"].astype(np.float32)
                + outs[2 * b + 1]["ypart"].astype(np.float32) + bp)
        am[b] = (outs[2 * b]["apart"].astype(np.float32)
                 + outs[2 * b + 1]["apart"].astype(np.float32)) * (1.0 / H)
    return (y, am)


# revision 3
# speedup vs baseline: 1.1723x; 1.1723x over previous
"""Cross-attention kernel for Trainium2 (8 NeuronCores, Bass/Tile).

Sharding: core c handles batch b = c//2 and head-group hg = c%2 (8 of 16
heads).  Each core computes, for its (b, hg):
  - k/v/q projections in f16 (weights column-sliced per head group)
  - per-head masked softmax attention: scores in PSUM f32 (mask folded in
    via an identity-matmul accumulation), exp+row-sum on the scalar engine,
    probs normalized to f16 on the vector engine (tensor_scalar by 1/Z)
  - att-mean partial: in-place pairwise tree sum of the normalized probs,
    split between the vector and gpsimd engines; host applies the 1/16
  - y partial: normalized probs transposed via two xbar DMAs (sync engine),
    AV + output projection on the PE -> host adds halves + bias
The per-chunk software pipeline runs AV/proj one chunk behind the scores
so every engine (PE/ACT/DVE/gpsimd/sync) stays busy concurrently.
"""

import os
import sys

sys.path.insert(0, "/opt/trn_rl_repo")

import numpy as np

import concourse.bass as bass
import concourse.tile as tile
from concourse import mybir
import concourse.bass_utils as bu

# ---------------------------------------------------------------- constants
B, T, TE, C = 4, 2048, 1024, 1024
H = 16          # total heads
HG = 8          # heads per group (per core)
D = 64          # head dim
KT = 8          # contraction tiles for Q/K (bias added in PSUM->SBUF copy)
KTV = 9         # V keeps a ones-row for its bias (v is s-major, bias on free dim)
CV_PAD = KTV * 128
EXPB = -2.0     # constant exp bias (cancels in softmax, guards fp16 overflow)
NEG = -30000.0  # additive mask value (exp underflows to exactly 0)
N_CORES = 8

f32 = mybir.dt.float32
f16 = mybir.dt.float16


def _split_waits(nc, max_waits=1):
    """walrus in this container accepts at most one sync-wait command per
    instruction; hoist extra waits onto preceding same-engine NoOps."""
    import bass_rust

    ctr = 0
    for f in nc.m.functions:
        for blk in f.blocks:
            il = list(blk.instructions)
            out = []
            changed = False
            for inst in il:
                si = inst.sync_info
                if si is not None and si.on_wait and len(si.on_wait) > max_waits:
                    waits = list(si.on_wait)
                    for w in waits[:-max_waits]:
                        ctr += 1
                        nop = mybir.InstNoOp(name=f"waitsplit_{ctr}", ins=[], outs=[])
                        nop.engine = inst.engine
                        nop.sync_info = bass_rust.SyncInfo(on_wait=[w], on_update=[])
                        out.append(nop)
                    inst.sync_info = bass_rust.SyncInfo(
                        on_wait=waits[-max_waits:],
                        on_update=list(si.on_update) if si.on_update else [],
                    )
                    changed = True
                out.append(inst)
            if changed:
                blk.instructions = out


def _build_program():
    nc = bass.Bass("TRN2", target_bir_lowering=False, debug=False)

    xta_d = nc.declare_dram_parameter("xta", [C, T], f16, isOutput=False)
    eta_d = nc.declare_dram_parameter("eta", [CV_PAD, TE], f16, isOutput=False)
    wq_d = nc.declare_dram_parameter("wqta", [C, 512], f16, isOutput=False)
    wk_d = nc.declare_dram_parameter("wkta", [C, 512], f16, isOutput=False)
    wv_d = nc.declare_dram_parameter("wvta", [CV_PAD, 512], f16, isOutput=False)
    wp_d = nc.declare_dram_parameter("wpt", [512, C], f16, isOutput=False)
    bqk_d = nc.declare_dram_parameter("bqk", [128, 8], f32, isOutput=False)
    mneg_d = nc.declare_dram_parameter("mneg", [T, TE], f16, isOutput=False)
    idh_d = nc.declare_dram_parameter("identh", [128, 128], f16, isOutput=False)
    y_d = nc.declare_dram_parameter("ypart", [T, C], f16, isOutput=True)
    a_d = nc.declare_dram_parameter("apart", [T, TE], f16, isOutput=True)

    xta_r = xta_d.rearrange("(kt p) n -> p kt n", p=128)
    eta_r = eta_d.rearrange("(kt p) n -> p kt n", p=128)
    wq_r = wq_d.rearrange("(kt p) n -> p kt n", p=128)
    wk_r = wk_d.rearrange("(kt p) n -> p kt n", p=128)
    wv_r = wv_d.rearrange("(kt p) n -> p kt n", p=128)
    wp_r = wp_d.rearrange("(kt p) n -> p kt n", p=128)
    mneg_r = mneg_d.rearrange("(tc p) s -> p tc s", p=128)

    with tile.TileContext(nc) as tc:
        with tc.tile_pool(name="persist", bufs=1) as persist:
            qT = persist.tile([128, 4, T], f16, tag="qT")
            kT = persist.tile([128, 4, TE], f16, tag="kT")
            vsb = persist.tile([128, 8, 512], f16, tag="vsb")
            wp = persist.tile([128, 4, C], f16, tag="wp")
            mka = persist.tile([128, 16, TE], f16, tag="mka")
            idh = persist.tile([128, 128], f16, tag="idh")
            bqk = persist.tile([128, 8], f32, tag="bqk")
            eb = persist.tile([128, 1], f32, tag="eb")

            nc.scalar.dma_start(out=idh, in_=idh_d[:, :])
            nc.scalar.dma_start(out=bqk, in_=bqk_d[:, :])
            nc.vector.memset(eb, EXPB)
            nc.scalar.dma_start(out=wp, in_=wp_r[:, :, :])

            # ---------------- stage A: projections (K, V, then Q) ----------
            psA_ctx = tc.tile_pool(name="psA", bufs=2, space="PSUM")
            psA = psA_ctx.__enter__()
            with tc.tile_pool(name="wkvpool", bufs=1) as wkvpool:
                wk = wkvpool.tile([128, KT, 512], f16, tag="wk")
                wv = wkvpool.tile([128, KTV, 512], f16, tag="wv")
                nc.scalar.dma_start(out=wk, in_=wk_r[:, :, :])
                nc.scalar.dma_start(out=wv, in_=wv_r[:, :, :])
                with tc.tile_pool(name="epool", bufs=2) as epool:
                    for sh in range(2):
                        ssl = slice(sh * 512, (sh + 1) * 512)
                        et = epool.tile([128, KTV, 512], f16, tag="et")
                        nc.sync.dma_start(out=et, in_=eta_r[:, :, ssl])
                        for pt in range(4):
                            ps = psA.tile([128, 512], f32, tag="psA")
                            for kt in range(KT):
                                nc.tensor.matmul(
                                    ps[:, :],
                                    wk[:, kt, pt * 128:(pt + 1) * 128],
                                    et[:, kt, :],
                                    start=(kt == 0), stop=(kt == KT - 1),
                                )
                            nc.scalar.activation(
                                kT[:, pt, ssl], ps[:, :],
                                mybir.ActivationFunctionType.Identity,
                                bias=bqk[:, 4 + pt:5 + pt],
                            )
                        for st4 in range(4):
                            ps = psA.tile([128, 512], f32, tag="psA")
                            for kt in range(KTV):
                                nc.tensor.matmul(
                                    ps[:, :],
                                    et[:, kt, st4 * 128:(st4 + 1) * 128],
                                    wv[:, kt, :],
                                    start=(kt == 0), stop=(kt == KTV - 1),
                                )
                            nc.scalar.copy(vsb[:, sh * 4 + st4, :], ps[:, :])

            with tc.tile_pool(name="wqpool", bufs=1) as wqpool:
                wq = wqpool.tile([128, KT, 512], f16, tag="wq")
                nc.scalar.dma_start(out=wq, in_=wq_r[:, :, :])
                with tc.tile_pool(name="xpool", bufs=2) as xpool:
                    for Tc in range(4):
                        tsl = slice(Tc * 512, (Tc + 1) * 512)
                        xt = xpool.tile([128, KT, 512], f16, tag="xt")
                        nc.sync.dma_start(out=xt, in_=xta_r[:, :, tsl])
                        for pt in range(4):
                            ps = psA.tile([128, 512], f32, tag="psA")
                            for kt in range(KT):
                                nc.tensor.matmul(
                                    ps[:, :],
                                    wq[:, kt, pt * 128:(pt + 1) * 128],
                                    xt[:, kt, :],
                                    start=(kt == 0), stop=(kt == KT - 1),
                                )
                            nc.scalar.activation(
                                qT[:, pt, tsl], ps[:, :],
                                mybir.ActivationFunctionType.Identity,
                                bias=bqk[:, pt:pt + 1],
                            )
            psA_ctx.__exit__(None, None, None)

            # mask preload: consumed chunk-by-chunk in stage B
            for q in range(4):
                nc.sync.dma_start(out=mka[:, 4 * q:4 * (q + 1), :],
                                  in_=mneg_r[:, 4 * q:4 * (q + 1), :])

            # ---------------- stage B: attention ----------------
            with (
                tc.tile_pool(name="spool", bufs=2, space="PSUM") as spool,
                tc.tile_pool(name="ypool", bufs=2, space="PSUM") as ypool,
                tc.tile_pool(name="ppool", bufs=2, space="PSUM") as ppool,
                tc.tile_pool(name="attEpool", bufs=2) as attEpool,
                tc.tile_pool(name="tmppool", bufs=3) as tmppool,
                tc.tile_pool(name="attTpool", bufs=2) as attTpool,
                tc.tile_pool(name="zpoolZ", bufs=8) as zpoolZ,
                tc.tile_pool(name="zpoolR", bufs=8) as zpoolR,
                tc.tile_pool(name="accpool", bufs=2) as accpool,
                tc.tile_pool(name="ytpool", bufs=2) as ytpool,
                tc.tile_pool(name="opool", bufs=2) as opool,
            ):
                def scores_pair(tci, hp, aE, tmp):
                    """QK+mask matmuls, exp, 1/Z and prob-normalize for heads
                    (2*hp, 2*hp+1) of chunk tci."""
                    tsl = slice(tci * 128, (tci + 1) * 128)
                    S0 = spool.tile([128, TE], f32, tag="S")
                    S1 = spool.tile([128, TE], f32, tag="S")
                    for sh in range(2):
                        ssl = slice(sh * 512, (sh + 1) * 512)
                        for h2, S in ((0, S0), (1, S1)):
                            hrow = slice(h2 * 64, (h2 + 1) * 64)
                            nc.tensor.matmul(
                                S[:, ssl],
                                qT[hrow, hp, tsl],
                                kT[hrow, hp, ssl],
                                start=True, stop=False,
                                tile_position=(h2 * 64, 0),
                            )
                    for h2, S in ((0, S0), (1, S1)):
                        for sh in range(2):
                            ssl = slice(sh * 512, (sh + 1) * 512)
                            nc.tensor.matmul(
                                S[:, ssl], idh[:, :], mka[:, tci, ssl],
                                start=False, stop=True,
                            )
                    Zp = zpoolZ.tile([128, 2], f32, tag="Zp")
                    for h2, S in ((0, S0), (1, S1)):
                        h = hp * 2 + h2
                        nc.scalar.activation(
                            aE[:, h, :], S[:, :],
                            mybir.ActivationFunctionType.Exp,
                            bias=eb[:, 0:1],
                            accum_out=Zp[:, h2:h2 + 1],
                        )
                    rcp = zpoolR.tile([128, 2], f32, tag="rcp")
                    nc.vector.reciprocal(rcp[:, :], Zp[:, :])
                    for h2 in range(2):
                        h = hp * 2 + h2
                        nc.vector.tensor_scalar_mul(
                            tmp[:, h, :], aE[:, h, :], rcp[:, h2:h2 + 1])

                def av_pair(hp, aT, yts):
                    """AV matmuls + f16 cast for heads (2*hp, 2*hp+1)."""
                    yps = ypool.tile([128, 128], f32, tag="yps")
                    for st in range(8):
                        for h2 in range(2):
                            h = hp * 2 + h2
                            nc.tensor.matmul(
                                yps[h2 * 64:(h2 + 1) * 64, :],
                                vsb[:, st, h * 64:(h + 1) * 64],
                                aT[:, h * 8 + st, :],
                                start=(st == 0), stop=(st == 7),
                                tile_position=(0, h2 * 64),
                            )
                    nc.vector.tensor_copy(yts[:, hp, :], yps[:, :])

                def proj_out(tci, yts):
                    """Output projection + DMA for chunk tci."""
                    tsl = slice(tci * 128, (tci + 1) * 128)
                    oc = opool.tile([128, C], f16, tag="oc")
                    for ch in range(2):
                        csl = slice(ch * 512, (ch + 1) * 512)
                        pps = ppool.tile([128, 512], f32, tag="pps")
                        for kt in range(4):
                            nc.tensor.matmul(
                                pps[:, :], yts[:, kt, :], wp[:, kt, csl],
                                start=(kt == 0), stop=(kt == 3),
                            )
                        nc.vector.tensor_copy(oc[:, csl], pps[:, :])
                    nc.gpsimd.dma_start(out=y_d[tsl, :], in_=oc[:, :])

                def att_mean(tci, tmp):
                    """Pairwise in-place tree sum of normalized probs; the
                    low half on the vector engine, high half on gpsimd."""
                    tsl = slice(tci * 128, (tci + 1) * 128)
                    nc.vector.tensor_add(tmp[:, 0, :], tmp[:, 0, :], tmp[:, 1, :])
                    nc.vector.tensor_add(tmp[:, 2, :], tmp[:, 2, :], tmp[:, 3, :])
                    nc.vector.tensor_add(tmp[:, 0, :], tmp[:, 0, :], tmp[:, 2, :])
                    nc.gpsimd.tensor_add(tmp[:, 4, :], tmp[:, 4, :], tmp[:, 5, :])
                    nc.gpsimd.tensor_add(tmp[:, 6, :], tmp[:, 6, :], tmp[:, 7, :])
                    nc.gpsimd.tensor_add(tmp[:, 4, :], tmp[:, 4, :], tmp[:, 6, :])
                    acc = accpool.tile([128, TE], f16, tag="acc")
                    nc.vector.tensor_add(acc[:, :], tmp[:, 0, :], tmp[:, 4, :])
                    nc.gpsimd.dma_start(out=a_d[tsl, :], in_=acc[:, :])

                prev = None   # (tci, aT, yts) one chunk behind
                for tci in range(16):
                    aE = attEpool.tile([128, HG, TE], f16, tag="aE")
                    tmp = tmppool.tile([128, HG, TE], f16, tag="tmp")
                    aT = attTpool.tile([128, HG * 8, 128], f16, tag="aT")
                    yts = ytpool.tile([128, 4, 128], f16, tag="yts")
                    for hp in range(4):
                        scores_pair(tci, hp, aE, tmp)
                        if hp == 1:
                            nc.sync.dma_start_transpose(
                                aT[:, 0:32, :], tmp[:, 0:4, :])
                        if prev is not None:
                            av_pair(hp, prev[1], prev[2])
                    nc.sync.dma_start_transpose(aT[:, 32:64, :], tmp[:, 4:8, :])
                    if prev is not None:
                        proj_out(prev[0], prev[2])
                        att_mean(prev[0], prev[3])
                    prev = (tci, aT, yts, tmp)
                tci, aT, yts, tmp = prev
                for hp in range(4):
                    av_pair(hp, aT, yts)
                proj_out(tci, yts)
                att_mean(tci, tmp)

    _split_waits(nc)
    return nc


_PROGRAM = None


def _get_program():
    global _PROGRAM
    if _PROGRAM is None:
        _PROGRAM = _build_program()
    return _PROGRAM


def _host_inputs(x, encoder_output, mask, Wq, bq, Wk, bk, Wv, bv, Wp, bp):
    """Build the 8 per-core input maps."""
    x = np.asarray(x, np.float32)
    enc = np.asarray(encoder_output, np.float32)
    mask = np.asarray(mask)
    scale = 1.0 / np.sqrt(D)
    identh = np.eye(128, dtype=np.float16)

    in_maps = []
    for c in range(N_CORES):
        b, hg = c // 2, c % 2
        hsl = slice(hg * 512, (hg + 1) * 512)

        xta = np.ascontiguousarray(x[b].T, dtype=np.float16)
        eta = np.zeros((CV_PAD, TE), np.float16)
        eta[:C] = enc[b].T
        eta[C] = 1.0

        wqta = np.ascontiguousarray(
            (np.asarray(Wq, np.float32)[hsl] * scale).T, dtype=np.float16)
        wkta = np.ascontiguousarray(
            np.asarray(Wk, np.float32)[hsl].T, dtype=np.float16)
        wvta = np.zeros((CV_PAD, 512), np.float16)
        wvta[:C] = np.asarray(Wv, np.float32)[hsl].T
        wvta[C] = np.asarray(bv, np.float32)[hsl]
        wpt = np.ascontiguousarray(
            np.asarray(Wp, np.float32)[:, hsl].T, dtype=np.float16)

        bqk = np.empty((128, 8), np.float32)
        bqk[:, 0:4] = (np.asarray(bq, np.float32)[hsl] * scale).reshape(4, 128).T
        bqk[:, 4:8] = np.asarray(bk, np.float32)[hsl].reshape(4, 128).T

        mneg = (mask[b].astype(np.float16)) * np.float16(NEG)

        in_maps.append({
            "xta": xta,
            "eta": eta,
            "wqta": wqta,
            "wkta": wkta,
            "wvta": wvta,
            "wpt": wpt,
            "bqk": bqk,
            "mneg": mneg,
            "identh": identh,
        })
    return in_maps


def kernel(x, encoder_output, mask, Wq, bq, Wk, bk, Wv, bv, Wp, bp):
    nc = _get_program()
    in_maps = _host_inputs(x, encoder_output, mask, Wq, bq, Wk, bk, Wv, bv, Wp, bp)
    trace = bool(int(os.environ.get("KERNEL_TRACE", "0")))
    res = bu.run_bass_kernel_spmd(nc, in_maps, list(range(N_CORES)), trace=trace)
    if trace:
        kernel.last_exec_time_ns = res.exec_time_ns
        kernel.last_profile = res
    outs = res.results

    bp = np.asarray(bp, np.float32)
    y = np.empty((B, T, C), np.float32)
    am = np.empty((B, T, TE), np.float32)
    for b in range(B):
        y[b] = (outs[2 * b]["ypart"].astype(np.float32)
                + outs[2 * b + 1]["ypart"].astype(np.float32) + bp)
        am[b] = (outs[2 * b]["apart"].astype(np.float32)
                 + outs[2 * b + 1]["apart"].astype(np.float32)) * (1.0 / H)
    return (y, am)


# revision 5
# speedup vs baseline: 1.3360x; 1.1397x over previous
"""Cross-attention kernel for Trainium2 (8 NeuronCores, Bass/Tile).

Sharding: core c handles batch b = c//2 and head-group hg = c%2 (8 of 16
heads).  Per (b, hg): k/v/q projections in f16; per-head masked softmax
(scores in PSUM f32, mask folded in via identity-matmul accumulation, exp +
row-sum on ACT, probs normalized to f16 on DVE); att-mean partial as a
pairwise tree sum (DVE low half @2x f16, gpsimd high half); AV on the
transposed normalized probs (two xbar DMAs per chunk on sync) + output
projection -> host adds head-group halves + bias, and scales att-mean by
1/16.

Steady-state engine budget per 128-row t-chunk (x16): ACT ~9.6us (exp
stream, kept continuous by a 3-buffer PSUM score rotation), PE ~9us
(QK+mask+AV+proj), DVE ~9us, sync ~9.7us (transposes), gpsimd ~8us.
AV/proj/att-mean run one chunk behind the score/exp pipeline.
"""

import os
import sys

sys.path.insert(0, "/opt/trn_rl_repo")

import numpy as np

import concourse.bass as bass
import concourse.tile as tile
from concourse import mybir
import concourse.bass_utils as bu

# ---------------------------------------------------------------- constants
B, T, TE, C = 4, 2048, 1024, 1024
H = 16          # total heads
HG = 8          # heads per group (per core)
D = 64          # head dim
KT = 8          # contraction tiles for Q/K (bias added in PSUM->SBUF copy)
KTV = 9         # V keeps a ones-row for its bias (v is s-major, bias on free dim)
CV_PAD = KTV * 128
EXPB = -2.0     # constant exp bias (cancels in softmax, guards fp16 overflow)
NEG = -30000.0  # additive mask value (exp underflows to exactly 0)
N_CORES = 8

f32 = mybir.dt.float32
f16 = mybir.dt.float16


def _split_waits(nc, max_waits=1):
    """walrus in this container accepts at most one sync-wait command per
    instruction; hoist extra waits onto preceding same-engine NoOps."""
    import bass_rust

    ctr = 0
    for f in nc.m.functions:
        for blk in f.blocks:
            il = list(blk.instructions)
            out = []
            changed = False
            for inst in il:
                si = inst.sync_info
                if si is not None and si.on_wait and len(si.on_wait) > max_waits:
                    waits = list(si.on_wait)
                    for w in waits[:-max_waits]:
                        ctr += 1
                        nop = mybir.InstNoOp(name=f"waitsplit_{ctr}", ins=[], outs=[])
                        nop.engine = inst.engine
                        nop.sync_info = bass_rust.SyncInfo(on_wait=[w], on_update=[])
                        out.append(nop)
                    inst.sync_info = bass_rust.SyncInfo(
                        on_wait=waits[-max_waits:],
                        on_update=list(si.on_update) if si.on_update else [],
                    )
                    changed = True
                out.append(inst)
            if changed:
                blk.instructions = out


def _build_program():
    nc = bass.Bass("TRN2", target_bir_lowering=False, debug=False)

    xta_d = nc.declare_dram_parameter("xta", [C, T], f16, isOutput=False)
    eta_d = nc.declare_dram_parameter("eta", [CV_PAD, TE], f16, isOutput=False)
    wq_d = nc.declare_dram_parameter("wqta", [C, 512], f16, isOutput=False)
    wk_d = nc.declare_dram_parameter("wkta", [C, 512], f16, isOutput=False)
    wv_d = nc.declare_dram_parameter("wvta", [CV_PAD, 512], f16, isOutput=False)
    wp_d = nc.declare_dram_parameter("wpt", [512, C], f16, isOutput=False)
    bqk_d = nc.declare_dram_parameter("bqk", [128, 8], f32, isOutput=False)
    mneg_d = nc.declare_dram_parameter("mneg", [T, TE], f16, isOutput=False)
    idh_d = nc.declare_dram_parameter("identh", [128, 128], f16, isOutput=False)
    y_d = nc.declare_dram_parameter("ypart", [T, C], f16, isOutput=True)
    a_d = nc.declare_dram_parameter("apart", [T, TE], f16, isOutput=True)

    xta_r = xta_d.rearrange("(kt p) n -> p kt n", p=128)
    eta_r = eta_d.rearrange("(kt p) n -> p kt n", p=128)
    wq_r = wq_d.rearrange("(kt p) n -> p kt n", p=128)
    wk_r = wk_d.rearrange("(kt p) n -> p kt n", p=128)
    wv_r = wv_d.rearrange("(kt p) n -> p kt n", p=128)
    wp_r = wp_d.rearrange("(kt p) n -> p kt n", p=128)
    mneg_r = mneg_d.rearrange("(tc p) s -> p tc s", p=128)

    with tile.TileContext(nc) as tc:
        with tc.tile_pool(name="persist", bufs=1) as persist:
            qT = persist.tile([128, 4, T], f16, tag="qT")
            kT = persist.tile([128, 4, TE], f16, tag="kT")
            vsb = persist.tile([128, 8, 512], f16, tag="vsb")
            wp = persist.tile([128, 4, C], f16, tag="wp")
            mka = persist.tile([128, 16, TE], f16, tag="mka")
            idh = persist.tile([128, 128], f16, tag="idh")
            bqk = persist.tile([128, 8], f32, tag="bqk")
            eb = persist.tile([128, 1], f32, tag="eb")

            nc.scalar.dma_start(out=idh, in_=idh_d[:, :])
            nc.scalar.dma_start(out=bqk, in_=bqk_d[:, :])
            nc.vector.memset(eb, EXPB)
            # mask preload rides the otherwise-idle gpsimd queue in stage A
            nc.gpsimd.dma_start(out=mka, in_=mneg_r[:, :, :])

            # ---------------- stage A: projections (K, V, then Q) ----------
            psA_ctx = tc.tile_pool(name="psA", bufs=2, space="PSUM")
            psA = psA_ctx.__enter__()
            with (
                tc.tile_pool(name="wkvpool", bufs=1) as wkvpool,
                tc.tile_pool(name="inpool", bufs=1) as inpool,
            ):
                wk = wkvpool.tile([128, KT, 512], f16, tag="wk")
                wv = wkvpool.tile([128, KTV, 512], f16, tag="wv")
                et = inpool.tile([128, KTV, TE], f16, tag="et")
                nc.scalar.dma_start(out=wk, in_=wk_r[:, :, :])
                nc.scalar.dma_start(out=wv, in_=wv_r[:, :, :])
                nc.sync.dma_start(out=et, in_=eta_r[:, :, :])
                xt = inpool.tile([128, KT, T], f16, tag="xt")
                wq = wkvpool.tile([128, KT, 512], f16, tag="wq")
                nc.scalar.dma_start(out=wq, in_=wq_r[:, :, :])
                nc.sync.dma_start(out=xt, in_=xta_r[:, :, :])
                nc.scalar.dma_start(out=wp, in_=wp_r[:, :, :])
                for sh in range(2):
                    ssl = slice(sh * 512, (sh + 1) * 512)
                    for pt in range(4):
                        ps = psA.tile([128, 512], f32, tag="psA")
                        for kt in range(KT):
                            nc.tensor.matmul(
                                ps[:, :],
                                wk[:, kt, pt * 128:(pt + 1) * 128],
                                et[:, kt, ssl],
                                start=(kt == 0), stop=(kt == KT - 1),
                            )
                        nc.scalar.activation(
                            kT[:, pt, ssl], ps[:, :],
                            mybir.ActivationFunctionType.Identity,
                            bias=bqk[:, 4 + pt:5 + pt],
                        )
                    for st4 in range(4):
                        st = sh * 4 + st4
                        ps = psA.tile([128, 512], f32, tag="psA")
                        for kt in range(KTV):
                            nc.tensor.matmul(
                                ps[:, :],
                                et[:, kt, st * 128:(st + 1) * 128],
                                wv[:, kt, :],
                                start=(kt == 0), stop=(kt == KTV - 1),
                            )
                        nc.scalar.copy(vsb[:, st, :], ps[:, :])
                for Tc in range(4):
                    tsl = slice(Tc * 512, (Tc + 1) * 512)
                    for pt in range(4):
                        ps = psA.tile([128, 512], f32, tag="psA")
                        for kt in range(KT):
                            nc.tensor.matmul(
                                ps[:, :],
                                wq[:, kt, pt * 128:(pt + 1) * 128],
                                xt[:, kt, tsl],
                                start=(kt == 0), stop=(kt == KT - 1),
                            )
                        nc.scalar.activation(
                            qT[:, pt, tsl], ps[:, :],
                            mybir.ActivationFunctionType.Identity,
                            bias=bqk[:, pt:pt + 1],
                        )
            psA_ctx.__exit__(None, None, None)

            # ---------------- stage B: attention ----------------
            with (
                tc.tile_pool(name="spool", bufs=3, space="PSUM") as spool,
                tc.tile_pool(name="ypool", bufs=1, space="PSUM") as ypool,
                tc.tile_pool(name="ppool", bufs=1, space="PSUM") as ppool,
                tc.tile_pool(name="attEpool", bufs=2) as attEpool,
                tc.tile_pool(name="tmppool", bufs=2) as tmppool,
                tc.tile_pool(name="attTpool", bufs=2) as attTpool,
                tc.tile_pool(name="zpoolZ", bufs=8) as zpoolZ,
                tc.tile_pool(name="zpoolR", bufs=8) as zpoolR,
                tc.tile_pool(name="treepool", bufs=2) as treepool,
                tc.tile_pool(name="accpool", bufs=2) as accpool,
                tc.tile_pool(name="ytpool", bufs=2) as ytpool,
                tc.tile_pool(name="opool", bufs=2) as opool,
            ):
                def scores_pair(tci, p, aE, tmp):
                    """QK+mask matmuls, exp, 1/Z and prob-normalize for heads
                    (2p, 2p+1) of chunk tci."""
                    tsl = slice(tci * 128, (tci + 1) * 128)
                    S0 = spool.tile([128, TE], f32, tag="S")
                    S1 = spool.tile([128, TE], f32, tag="S")
                    for sh in range(2):
                        ssl = slice(sh * 512, (sh + 1) * 512)
                        for h2, S in ((0, S0), (1, S1)):
                            hrow = slice(h2 * 64, (h2 + 1) * 64)
                            nc.tensor.matmul(
                                S[:, ssl],
                                qT[hrow, p, tsl],
                                kT[hrow, p, ssl],
                                start=True, stop=False,
                                tile_position=(h2 * 64, 0),
                            )
                    for h2, S in ((0, S0), (1, S1)):
                        for sh in range(2):
                            ssl = slice(sh * 512, (sh + 1) * 512)
                            nc.tensor.matmul(
                                S[:, ssl], idh[:, :], mka[:, tci, ssl],
                                start=False, stop=True,
                            )
                    Zp = zpoolZ.tile([128, 2], f32, tag="Zp")
                    for h2, S in ((0, S0), (1, S1)):
                        nc.scalar.activation(
                            aE[:, p * 2 + h2, :], S[:, :],
                            mybir.ActivationFunctionType.Exp,
                            bias=eb[:, 0:1],
                            accum_out=Zp[:, h2:h2 + 1],
                        )
                    rcp = zpoolR.tile([128, 2], f32, tag="rcp")
                    nc.vector.reciprocal(rcp[:, :], Zp[:, :])
                    for h2 in range(2):
                        h = p * 2 + h2
                        nc.vector.tensor_scalar_mul(
                            tmp[:, h, :], aE[:, h, :], rcp[:, h2:h2 + 1])

                def av_pair(p, aT, yps4):
                    """AV matmuls for heads (2p, 2p+1) into yps4[:, p, :]."""
                    for st in range(8):
                        for h2 in range(2):
                            h = p * 2 + h2
                            nc.tensor.matmul(
                                yps4[h2 * 64:(h2 + 1) * 64, p, :],
                                vsb[:, st, h * 64:(h + 1) * 64],
                                aT[:, h * 8 + st, :],
                                start=(st == 0), stop=(st == 7),
                                tile_position=(0, h2 * 64),
                            )

                def proj_out(tci, yps4):
                    """f16 cast, output projection + DMA for chunk tci."""
                    tsl = slice(tci * 128, (tci + 1) * 128)
                    yts = ytpool.tile([128, 4, 128], f16, tag="yts")
                    nc.vector.tensor_copy(yts[:, :, :], yps4[:, :, :])
                    oc = opool.tile([128, C], f16, tag="oc")
                    for ch in range(2):
                        csl = slice(ch * 512, (ch + 1) * 512)
                        pps = ppool.tile([128, 512], f32, tag="pps")
                        for kt in range(4):
                            nc.tensor.matmul(
                                pps[:, :], yts[:, kt, :], wp[:, kt, csl],
                                start=(kt == 0), stop=(kt == 3),
                            )
                        nc.vector.tensor_copy(oc[:, csl], pps[:, :])
                    nc.gpsimd.dma_start(out=y_d[tsl, :], in_=oc[:, :])

                def att_tree(tmp, tree):
                    """First tree level+ for att-mean: DVE sums heads 0-3 at
                    2x f16; gpsimd sums heads 4-7."""
                    t01, t23, tA, t45, t67, tB = tree
                    nc.vector.tensor_add(t01, tmp[:, 0, :], tmp[:, 1, :])
                    nc.vector.tensor_add(t23, tmp[:, 2, :], tmp[:, 3, :])
                    nc.vector.tensor_add(tA, t01, t23)
                    nc.gpsimd.tensor_add(t45, tmp[:, 4, :], tmp[:, 5, :])
                    nc.gpsimd.tensor_add(t67, tmp[:, 6, :], tmp[:, 7, :])
                    nc.gpsimd.tensor_add(tB, t45, t67)

                def att_final(tci, tree):
                    tsl = slice(tci * 128, (tci + 1) * 128)
                    acc = accpool.tile([128, TE], f16, tag="acc")
                    nc.vector.tensor_add(acc[:, :], tree[2], tree[5])
                    nc.gpsimd.dma_start(out=a_d[tsl, :], in_=acc[:, :])

                def tree_tiles():
                    return [treepool.tile([128, TE], f16, tag=t, name=t)
                            for t in ("t01", "t23", "tA", "t45", "t67", "tB")]

                prev = None   # (tci, aT, tmp, tree, yps4) one chunk behind
                for tci in range(16):
                    aE = attEpool.tile([128, HG, TE], f16, tag="aE")
                    tmp = tmppool.tile([128, HG, TE], f16, tag="tmp")
                    aT = attTpool.tile([128, HG * 8, 128], f16, tag="aT")
                    if prev is not None:
                        att_tree(prev[2], prev[3])
                        yps4 = ypool.tile([128, 4, 128], f32, tag="yps4")
                    for p in range(4):
                        scores_pair(tci, p, aE, tmp)
                        if p == 1:
                            nc.sync.dma_start_transpose(
                                aT[:, 0:32, :], tmp[:, 0:4, :])
                        if prev is not None:
                            av_pair(p, prev[1], yps4)
                            if p == 1:
                                att_final(prev[0], prev[3])
                    nc.sync.dma_start_transpose(aT[:, 32:64, :], tmp[:, 4:8, :])
                    tree = tree_tiles()
                    if prev is not None:
                        proj_out(prev[0], yps4)
                    prev = (tci, aT, tmp, tree)
                # flush the last chunk
                tci, aT, tmp, tree = prev
                att_tree(tmp, tree)
                yps4 = ypool.tile([128, 4, 128], f32, tag="yps4")
                for p in range(4):
                    av_pair(p, aT, yps4)
                att_final(tci, tree)
                proj_out(tci, yps4)

    _split_waits(nc)
    return nc


_PROGRAM = None


def _get_program():
    global _PROGRAM
    if _PROGRAM is None:
        _PROGRAM = _build_program()
    return _PROGRAM


def _host_inputs(x, encoder_output, mask, Wq, bq, Wk, bk, Wv, bv, Wp, bp):
    """Build the 8 per-core input maps."""
    x = np.asarray(x, np.float32)
    enc = np.asarray(encoder_output, np.float32)
    mask = np.asarray(mask)
    scale = 1.0 / np.sqrt(D)
    identh = np.eye(128, dtype=np.float16)

    in_maps = []
    for c in range(N_CORES):
        b, hg = c // 2, c % 2
        hsl = slice(hg * 512, (hg + 1) * 512)

        xta = np.ascontiguousarray(x[b].T, dtype=np.float16)
        eta = np.zeros((CV_PAD, TE), np.float16)
        eta[:C] = enc[b].T
        eta[C] = 1.0

        wqta = np.ascontiguousarray(
            (np.asarray(Wq, np.float32)[hsl] * scale).T, dtype=np.float16)
        wkta = np.ascontiguousarray(
            np.asarray(Wk, np.float32)[hsl].T, dtype=np.float16)
        wvta = np.zeros((CV_PAD, 512), np.float16)
        wvta[:C] = np.asarray(Wv, np.float32)[hsl].T
        wvta[C] = np.asarray(bv, np.float32)[hsl]
        wpt = np.ascontiguousarray(
            np.asarray(Wp, np.float32)[:, hsl].T, dtype=np.float16)

        bqk = np.empty((128, 8), np.float32)
        bqk[:, 0:4] = (np.asarray(bq, np.float32)[hsl] * scale).reshape(4, 128).T
        bqk[:, 4:8] = np.asarray(bk, np.float32)[hsl].reshape(4, 128).T

        mneg = (mask[b].astype(np.float16)) * np.float16(NEG)

        in_maps.append({
            "xta": xta,
            "eta": eta,
            "wqta": wqta,
            "wkta": wkta,
            "wvta": wvta,
            "wpt": wpt,
            "bqk": bqk,
            "mneg": mneg,
            "identh": identh,
        })
    return in_maps


def kernel(x, encoder_output, mask, Wq, bq, Wk, bk, Wv, bv, Wp, bp):
    nc = _get_program()
    in_maps = _host_inputs(x, encoder_output, mask, Wq, bq, Wk, bk, Wv, bv, Wp, bp)
    trace = bool(int(os.environ.get("KERNEL_TRACE", "0")))
    res = bu.run_bass_kernel_spmd(nc, in_maps, list(range(N_CORES)), trace=trace)
    if trace:
        kernel.last_exec_time_ns = res.exec_time_ns
        kernel.last_profile = res
    outs = res.results

    bp = np.asarray(bp, np.float32)
    y = np.empty((B, T, C), np.float32)
    am = np.empty((B, T, TE), np.float32)
    for b in range(B):
        y[b] = (outs[2 * b]["ypart"].astype(np.float32)
                + outs[2 * b + 1]["ypart"].astype(np.float32) + bp)
        am[b] = (outs[2 * b]["apart"].astype(np.float32)
                 + outs[2 * b + 1]["apart"].astype(np.float32)) * (1.0 / H)
    return (y, am)


# revision 6
# speedup vs baseline: 1.3998x; 1.0477x over previous
"""Cross-attention kernel for Trainium2 (8 NeuronCores, Bass/Tile).

Sharding: core c handles batch b = c//2 and head-group hg = c%2 (8 of 16
heads).  Per (b, hg): k/v/q projections in f16; per-head masked softmax
(scores in PSUM f32, mask folded in via an fp8 identity-matmul
accumulation, exp + row-sum on ACT, probs normalized to f16 on DVE);
att-mean partial as a pairwise tree sum (DVE low half @2x f16, gpsimd high
half); AV on the transposed normalized probs (two xbar DMAs per chunk on
sync) + output projection -> host adds head-group halves + bias, and
scales att-mean by 1/16.

Pipelining: the score/exp stream for chunk i runs concurrently with the
AV/proj of chunk i-2 (two behind, so the PE never takes a long stall that
would re-throttle the HAM clock gate) and the att-mean tree of chunk i-1.
Stage-A loads are chunked and priority-ordered so the first projection
matmul starts ~6us in, with the remaining ~10MB streaming underneath.
"""

import os
import sys

sys.path.insert(0, "/opt/trn_rl_repo")

import numpy as np
import ml_dtypes

import concourse.bass as bass
import concourse.tile as tile
from concourse import mybir
import concourse.bass_utils as bu

# ---------------------------------------------------------------- constants
B, T, TE, C = 4, 2048, 1024, 1024
H = 16          # total heads
HG = 8          # heads per group (per core)
D = 64          # head dim
KT = 8          # contraction tiles for Q/K (bias added in PSUM->SBUF copy)
KTV = 9         # V keeps a ones-row for its bias (v is s-major, bias on free dim)
CV_PAD = KTV * 128
EXPB = -2.0     # constant exp bias (cancels in softmax, guards fp16 overflow)
NEG = -240.0    # additive mask value, exactly representable in fp8 e4m3;
                # exp(s + NEG + EXPB) underflows to exactly 0 for |s| < 200
N_CORES = 8

f32 = mybir.dt.float32
f16 = mybir.dt.float16
f8 = mybir.dt.float8e4


def _split_waits(nc, max_waits=1):
    """walrus in this container accepts at most one sync-wait command per
    instruction; hoist extra waits onto preceding same-engine NoOps."""
    import bass_rust

    ctr = 0
    for f in nc.m.functions:
        for blk in f.blocks:
            il = list(blk.instructions)
            out = []
            changed = False
            for inst in il:
                si = inst.sync_info
                if si is not None and si.on_wait and len(si.on_wait) > max_waits:
                    waits = list(si.on_wait)
                    for w in waits[:-max_waits]:
                        ctr += 1
                        nop = mybir.InstNoOp(name=f"waitsplit_{ctr}", ins=[], outs=[])
                        nop.engine = inst.engine
                        nop.sync_info = bass_rust.SyncInfo(on_wait=[w], on_update=[])
                        out.append(nop)
                    inst.sync_info = bass_rust.SyncInfo(
                        on_wait=waits[-max_waits:],
                        on_update=list(si.on_update) if si.on_update else [],
                    )
                    changed = True
                out.append(inst)
            if changed:
                blk.instructions = out


def _build_program():
    nc = bass.Bass("TRN2", target_bir_lowering=False, debug=False)

    xta_d = nc.declare_dram_parameter("xta", [C, T], f16, isOutput=False)
    eta_d = nc.declare_dram_parameter("eta", [CV_PAD, TE], f16, isOutput=False)
    wq_d = nc.declare_dram_parameter("wqta", [C, 512], f16, isOutput=False)
    wk_d = nc.declare_dram_parameter("wkta", [C, 512], f16, isOutput=False)
    wv_d = nc.declare_dram_parameter("wvta", [CV_PAD, 512], f16, isOutput=False)
    wp_d = nc.declare_dram_parameter("wpt", [512, C], f16, isOutput=False)
    bqk_d = nc.declare_dram_parameter("bqk", [128, 8], f32, isOutput=False)
    mneg_d = nc.declare_dram_parameter("mneg", [T, TE], f8, isOutput=False)
    idh_d = nc.declare_dram_parameter("identh", [128, 128], f8, isOutput=False)
    y_d = nc.declare_dram_parameter("ypart", [T, C], f16, isOutput=True)
    a_d = nc.declare_dram_parameter("apart", [T, TE], f16, isOutput=True)

    xta_r = xta_d.rearrange("(kt p) n -> p kt n", p=128)
    eta_r = eta_d.rearrange("(kt p) n -> p kt n", p=128)
    wq_r = wq_d.rearrange("(kt p) n -> p kt n", p=128)
    wk_r = wk_d.rearrange("(kt p) n -> p kt n", p=128)
    wv_r = wv_d.rearrange("(kt p) n -> p kt n", p=128)
    wp_r = wp_d.rearrange("(kt p) n -> p kt n", p=128)
    mneg_r = mneg_d.rearrange("(tc p) s -> p tc s", p=128)

    with tile.TileContext(nc) as tc:
        with tc.tile_pool(name="persist", bufs=1) as persist:
            qT = persist.tile([128, 4, T], f16, tag="qT")
            kT = persist.tile([128, 4, TE], f16, tag="kT")
            vsb = persist.tile([128, 8, 512], f16, tag="vsb")
            wp = persist.tile([128, 4, C], f16, tag="wp")
            mka = persist.tile([128, 16, TE], f8, tag="mka")
            idh = persist.tile([128, 128], f8, tag="idh")
            bqk = persist.tile([128, 8], f32, tag="bqk")
            eb = persist.tile([128, 1], f32, tag="eb")

            # ---------------- stage A: projections (K, V, then Q) ----------
            # loads are chunked and ordered so the first matmul can start
            # after ~2MB (wk + half of enc^T) instead of the full ~12MB
            psA_ctx = tc.tile_pool(name="psA", bufs=2, space="PSUM")
            psA = psA_ctx.__enter__()
            with (
                tc.tile_pool(name="wkvpool", bufs=1) as wkvpool,
                tc.tile_pool(name="inpool", bufs=1) as inpool,
            ):
                wk = wkvpool.tile([128, KT, 512], f16, tag="wk")
                wv = wkvpool.tile([128, KTV, 512], f16, tag="wv")
                wq = wkvpool.tile([128, KT, 512], f16, tag="wq")
                et = inpool.tile([128, KTV, TE], f16, tag="et")
                xt = inpool.tile([128, KT, T], f16, tag="xt")

                nc.scalar.dma_start(out=idh, in_=idh_d[:, :])
                nc.scalar.dma_start(out=bqk, in_=bqk_d[:, :])
                nc.vector.memset(eb, EXPB)
                nc.scalar.dma_start(out=wk, in_=wk_r[:, :, :])
                nc.sync.dma_start(out=et[:, :, 0:512], in_=eta_r[:, :, 0:512])
                nc.gpsimd.dma_start(out=mka, in_=mneg_r[:, :, :])
                nc.scalar.dma_start(out=wv, in_=wv_r[:, :, :])
                nc.scalar.dma_start(out=wq, in_=wq_r[:, :, :])
                nc.sync.dma_start(out=et[:, :, 512:1024], in_=eta_r[:, :, 512:1024])
                nc.sync.dma_start(out=xt[:, :, 0:1024], in_=xta_r[:, :, 0:1024])
                nc.sync.dma_start(out=xt[:, :, 1024:2048], in_=xta_r[:, :, 1024:2048])
                nc.scalar.dma_start(out=wp, in_=wp_r[:, :, :])

                for sh in range(2):
                    ssl = slice(sh * 512, (sh + 1) * 512)
                    for pt in range(4):
                        ps = psA.tile([128, 512], f32, tag="psA")
                        for kt in range(KT):
                            nc.tensor.matmul(
                                ps[:, :],
                                wk[:, kt, pt * 128:(pt + 1) * 128],
                                et[:, kt, ssl],
                                start=(kt == 0), stop=(kt == KT - 1),
                            )
                        nc.scalar.activation(
                            kT[:, pt, ssl], ps[:, :],
                            mybir.ActivationFunctionType.Identity,
                            bias=bqk[:, 4 + pt:5 + pt],
                        )
                    for st4 in range(4):
                        st = sh * 4 + st4
                        ps = psA.tile([128, 512], f32, tag="psA")
                        for kt in range(KTV):
                            nc.tensor.matmul(
                                ps[:, :],
                                et[:, kt, st * 128:(st + 1) * 128],
                                wv[:, kt, :],
                                start=(kt == 0), stop=(kt == KTV - 1),
                            )
                        nc.scalar.copy(vsb[:, st, :], ps[:, :])
                for Tc in range(4):
                    tsl = slice(Tc * 512, (Tc + 1) * 512)
                    for pt in range(4):
                        ps = psA.tile([128, 512], f32, tag="psA")
                        for kt in range(KT):
                            nc.tensor.matmul(
                                ps[:, :],
                                wq[:, kt, pt * 128:(pt + 1) * 128],
                                xt[:, kt, tsl],
                                start=(kt == 0), stop=(kt == KT - 1),
                            )
                        nc.scalar.activation(
                            qT[:, pt, tsl], ps[:, :],
                            mybir.ActivationFunctionType.Identity,
                            bias=bqk[:, pt:pt + 1],
                        )
            psA_ctx.__exit__(None, None, None)

            # ---------------- stage B: attention ----------------
            with (
                tc.tile_pool(name="spool", bufs=3, space="PSUM") as spool,
                tc.tile_pool(name="ypool", bufs=1, space="PSUM") as ypool,
                tc.tile_pool(name="ppool", bufs=1, space="PSUM") as ppool,
                tc.tile_pool(name="attEpool", bufs=2) as attEpool,
                tc.tile_pool(name="tmppool", bufs=2) as tmppool,
                tc.tile_pool(name="attTpool", bufs=3) as attTpool,
                tc.tile_pool(name="zpoolZ", bufs=8) as zpoolZ,
                tc.tile_pool(name="zpoolR", bufs=8) as zpoolR,
                tc.tile_pool(name="treepool", bufs=2) as treepool,
                tc.tile_pool(name="accpool", bufs=2) as accpool,
                tc.tile_pool(name="ytpool", bufs=2) as ytpool,
                tc.tile_pool(name="opool", bufs=2) as opool,
            ):
                def scores_pair(tci, p, aE, tmp):
                    """QK+mask matmuls, exp, 1/Z and prob-normalize for heads
                    (2p, 2p+1) of chunk tci."""
                    tsl = slice(tci * 128, (tci + 1) * 128)
                    S0 = spool.tile([128, TE], f32, tag="S")
                    S1 = spool.tile([128, TE], f32, tag="S")
                    for sh in range(2):
                        ssl = slice(sh * 512, (sh + 1) * 512)
                        for h2, S in ((0, S0), (1, S1)):
                            hrow = slice(h2 * 64, (h2 + 1) * 64)
                            nc.tensor.matmul(
                                S[:, ssl],
                                qT[hrow, p, tsl],
                                kT[hrow, p, ssl],
                                start=True, stop=False,
                                tile_position=(h2 * 64, 0),
                            )
                    for h2, S in ((0, S0), (1, S1)):
                        for sh in range(2):
                            ssl = slice(sh * 512, (sh + 1) * 512)
                            nc.tensor.matmul(
                                S[:, ssl], idh[:, :], mka[:, tci, ssl],
                                start=False, stop=True,
                            )
                    Zp = zpoolZ.tile([128, 2], f32, tag="Zp")
                    for h2, S in ((0, S0), (1, S1)):
                        nc.scalar.activation(
                            aE[:, p * 2 + h2, :], S[:, :],
                            mybir.ActivationFunctionType.Exp,
                            bias=eb[:, 0:1],
                            accum_out=Zp[:, h2:h2 + 1],
                        )
                    rcp = zpoolR.tile([128, 2], f32, tag="rcp")
                    nc.vector.reciprocal(rcp[:, :], Zp[:, :])
                    for h2 in range(2):
                        h = p * 2 + h2
                        nc.vector.tensor_scalar_mul(
                            tmp[:, h, :], aE[:, h, :], rcp[:, h2:h2 + 1])

                def av_pair(p, aT, yps4):
                    """AV matmuls for heads (2p, 2p+1) into yps4[:, p, :]."""
                    for st in range(8):
                        for h2 in range(2):
                            h = p * 2 + h2
                            nc.tensor.matmul(
                                yps4[h2 * 64:(h2 + 1) * 64, p, :],
                                vsb[:, st, h * 64:(h + 1) * 64],
                                aT[:, h * 8 + st, :],
                                start=(st == 0), stop=(st == 7),
                                tile_position=(0, h2 * 64),
                            )

                def proj_out(tci, yps4):
                    """f16 cast, output projection + DMA for chunk tci."""
                    tsl = slice(tci * 128, (tci + 1) * 128)
                    yts = ytpool.tile([128, 4, 128], f16, tag="yts")
                    nc.vector.tensor_copy(yts[:, :, :], yps4[:, :, :])
                    oc = opool.tile([128, C], f16, tag="oc")
                    for ch in range(2):
                        csl = slice(ch * 512, (ch + 1) * 512)
                        pps = ppool.tile([128, 512], f32, tag="pps")
                        for kt in range(4):
                            nc.tensor.matmul(
                                pps[:, :], yts[:, kt, :], wp[:, kt, csl],
                                start=(kt == 0), stop=(kt == 3),
                            )
                        nc.vector.tensor_copy(oc[:, csl], pps[:, :])
                    nc.gpsimd.dma_start(out=y_d[tsl, :], in_=oc[:, :])

                def att_tree(tmp, tree):
                    """First tree levels for att-mean: DVE sums heads 0-3 at
                    2x f16; gpsimd sums heads 4-7."""
                    t01, t23, tA, t45, t67, tB = tree
                    nc.vector.tensor_add(t01, tmp[:, 0, :], tmp[:, 1, :])
                    nc.vector.tensor_add(t23, tmp[:, 2, :], tmp[:, 3, :])
                    nc.vector.tensor_add(tA, t01, t23)
                    nc.gpsimd.tensor_add(t45, tmp[:, 4, :], tmp[:, 5, :])
                    nc.gpsimd.tensor_add(t67, tmp[:, 6, :], tmp[:, 7, :])
                    nc.gpsimd.tensor_add(tB, t45, t67)

                def att_final(tci, tree):
                    tsl = slice(tci * 128, (tci + 1) * 128)
                    acc = accpool.tile([128, TE], f16, tag="acc")
                    nc.vector.tensor_add(acc[:, :], tree[2], tree[5])
                    nc.gpsimd.dma_start(out=a_d[tsl, :], in_=acc[:, :])

                def tree_tiles():
                    return [treepool.tile([128, TE], f16, tag=t, name=t)
                            for t in ("t01", "t23", "tA", "t45", "t67", "tB")]

                p1 = None   # (tci, aT, tmp, tree) one chunk behind
                p2 = None   # two chunks behind
                for tci in range(16):
                    aE = attEpool.tile([128, HG, TE], f16, tag="aE")
                    tmp = tmppool.tile([128, HG, TE], f16, tag="tmp")
                    aT = attTpool.tile([128, HG * 8, 128], f16, tag="aT")
                    if p1 is not None:
                        att_tree(p1[2], p1[3])
                    if p2 is not None:
                        yps4 = ypool.tile([128, 4, 128], f32, tag="yps4")
                    for p in range(4):
                        scores_pair(tci, p, aE, tmp)
                        if p == 1:
                            nc.sync.dma_start_transpose(
                                aT[:, 0:32, :], tmp[:, 0:4, :])
                        if p2 is not None:
                            av_pair(p, p2[1], yps4)
                            if p == 1:
                                att_final(p1[0], p1[3])
                        elif p == 1 and p1 is not None:
                            att_final(p1[0], p1[3])
                    nc.sync.dma_start_transpose(aT[:, 32:64, :], tmp[:, 4:8, :])
                    tree = tree_tiles()
                    if p2 is not None:
                        proj_out(p2[0], yps4)
                    p2 = p1
                    p1 = (tci, aT, tmp, tree)
                # flush the last two chunks
                att_tree(p1[2], p1[3])
                yps4 = ypool.tile([128, 4, 128], f32, tag="yps4")
                for p in range(4):
                    av_pair(p, p2[1], yps4)
                att_final(p1[0], p1[3])
                proj_out(p2[0], yps4)
                yps4b = ypool.tile([128, 4, 128], f32, tag="yps4")
                for p in range(4):
                    av_pair(p, p1[1], yps4b)
                proj_out(p1[0], yps4b)

    _split_waits(nc)
    return nc


_PROGRAM = None


def _get_program():
    global _PROGRAM
    if _PROGRAM is None:
        _PROGRAM = _build_program()
    return _PROGRAM


def _host_inputs(x, encoder_output, mask, Wq, bq, Wk, bk, Wv, bv, Wp, bp):
    """Build the 8 per-core input maps."""
    x = np.asarray(x, np.float32)
    enc = np.asarray(encoder_output, np.float32)
    mask = np.asarray(mask)
    scale = 1.0 / np.sqrt(D)
    f8np = ml_dtypes.float8_e4m3
    identh = np.eye(128, dtype=f8np)

    in_maps = []
    for c in range(N_CORES):
        b, hg = c // 2, c % 2
        hsl = slice(hg * 512, (hg + 1) * 512)

        xta = np.ascontiguousarray(x[b].T, dtype=np.float16)
        eta = np.zeros((CV_PAD, TE), np.float16)
        eta[:C] = enc[b].T
        eta[C] = 1.0

        wqta = np.ascontiguousarray(
            (np.asarray(Wq, np.float32)[hsl] * scale).T, dtype=np.float16)
        wkta = np.ascontiguousarray(
            np.asarray(Wk, np.float32)[hsl].T, dtype=np.float16)
        wvta = np.zeros((CV_PAD, 512), np.float16)
        wvta[:C] = np.asarray(Wv, np.float32)[hsl].T
        wvta[C] = np.asarray(bv, np.float32)[hsl]
        wpt = np.ascontiguousarray(
            np.asarray(Wp, np.float32)[:, hsl].T, dtype=np.float16)

        bqk = np.empty((128, 8), np.float32)
        bqk[:, 0:4] = (np.asarray(bq, np.float32)[hsl] * scale).reshape(4, 128).T
        bqk[:, 4:8] = np.asarray(bk, np.float32)[hsl].reshape(4, 128).T

        mneg = np.where(mask[b], np.float32(NEG), np.float32(0)).astype(f8np)

        in_maps.append({
            "xta": xta,
            "eta": eta,
            "wqta": wqta,
            "wkta": wkta,
            "wvta": wvta,
            "wpt": wpt,
            "bqk": bqk,
            "mneg": mneg,
            "identh": identh,
        })
    return in_maps


def kernel(x, encoder_output, mask, Wq, bq, Wk, bk, Wv, bv, Wp, bp):
    nc = _get_program()
    in_maps = _host_inputs(x, encoder_output, mask, Wq, bq, Wk, bk, Wv, bv, Wp, bp)
    trace = bool(int(os.environ.get("KERNEL_TRACE", "0")))
    res = bu.run_bass_kernel_spmd(nc, in_maps, list(range(N_CORES)), trace=trace)
    if trace:
        kernel.last_exec_time_ns = res.exec_time_ns
        kernel.last_profile = res
    outs = res.results

    bp = np.asarray(bp, np.float32)
    y = np.empty((B, T, C), np.float32)
    am = np.empty((B, T, TE), np.float32)
    for b in range(B):
        y[b] = (outs[2 * b]["ypart"].astype(np.float32)
                + outs[2 * b + 1]["ypart"].astype(np.float32) + bp)
        am[b] = (outs[2 * b]["apart"].astype(np.float32)
                 + outs[2 * b + 1]["apart"].astype(np.float32)) * (1.0 / H)
    return (y, am)


# revision 9
# speedup vs baseline: 1.4200x; 1.0144x over previous
"""Cross-attention kernel for Trainium2 (8 NeuronCores, Bass/Tile).

Sharding: core c handles batch b = c//2 and head-group hg = c%2 (8 of 16
heads).  Per (b, hg): k/v/q projections in f16; per-head masked softmax
(scores in PSUM f32, mask folded in via an fp8 identity-matmul
accumulation, exp + row-sum on ACT, probs normalized to f16 on DVE);
att-mean partial as a pairwise tree sum (DVE low half @2x f16, gpsimd high
half); AV on the transposed normalized probs (two xbar DMAs per chunk on
sync) + output projection -> host adds head-group halves + bias, and
scales att-mean by 1/16.

Pipelining: the score/exp stream for chunk i runs concurrently with the
AV/proj of chunk i-2 (two behind, so the PE never takes a long stall that
would re-throttle the HAM clock gate) and the att-mean tree of chunk i-1.
Stage-A loads are chunked and priority-ordered so the first projection
matmul starts ~6us in, with the remaining ~10MB streaming underneath.
"""

import os
import sys

sys.path.insert(0, "/opt/trn_rl_repo")

import numpy as np
import ml_dtypes

import concourse.bass as bass
import concourse.tile as tile
from concourse import mybir
import concourse.bass_utils as bu

# ---------------------------------------------------------------- constants
B, T, TE, C = 4, 2048, 1024, 1024
H = 16          # total heads
HG = 8          # heads per group (per core)
D = 64          # head dim
KT = 8          # contraction tiles for Q/K (bias added in PSUM->SBUF copy)
KTV = 9         # V keeps a ones-row for its bias (v is s-major, bias on free dim)
CV_PAD = KTV * 128
EXPB = -2.0     # constant exp bias (cancels in softmax, guards fp16 overflow)
NEG = -240.0    # additive mask value, exactly representable in fp8 e4m3;
                # exp(s + NEG + EXPB) underflows to exactly 0 for |s| < 200
N_CORES = 8

f32 = mybir.dt.float32
f16 = mybir.dt.float16
f8 = mybir.dt.float8e4


def _split_waits(nc, max_waits=1):
    """walrus in this container accepts at most one sync-wait command per
    instruction; hoist extra waits onto preceding same-engine NoOps."""
    import bass_rust

    ctr = 0
    for f in nc.m.functions:
        for blk in f.blocks:
            il = list(blk.instructions)
            out = []
            changed = False
            for inst in il:
                si = inst.sync_info
                if si is not None and si.on_wait and len(si.on_wait) > max_waits:
                    waits = list(si.on_wait)
                    for w in waits[:-max_waits]:
                        ctr += 1
                        nop = mybir.InstNoOp(name=f"waitsplit_{ctr}", ins=[], outs=[])
                        nop.engine = inst.engine
                        nop.sync_info = bass_rust.SyncInfo(on_wait=[w], on_update=[])
                        out.append(nop)
                    inst.sync_info = bass_rust.SyncInfo(
                        on_wait=waits[-max_waits:],
                        on_update=list(si.on_update) if si.on_update else [],
                    )
                    changed = True
                out.append(inst)
            if changed:
                blk.instructions = out


def _build_program():
    nc = bass.Bass("TRN2", target_bir_lowering=False, debug=False)

    xta_d = nc.declare_dram_parameter("xta", [C, T], f16, isOutput=False)
    eta_d = nc.declare_dram_parameter("eta", [CV_PAD, TE], f16, isOutput=False)
    wq_d = nc.declare_dram_parameter("wqta", [C, 512], f16, isOutput=False)
    wk_d = nc.declare_dram_parameter("wkta", [C, 512], f16, isOutput=False)
    wv_d = nc.declare_dram_parameter("wvta", [CV_PAD, 512], f16, isOutput=False)
    wp_d = nc.declare_dram_parameter("wpt", [512, C], f16, isOutput=False)
    bqk_d = nc.declare_dram_parameter("bqk", [128, 8], f32, isOutput=False)
    mneg_d = nc.declare_dram_parameter("mneg", [T, TE], f8, isOutput=False)
    idh_d = nc.declare_dram_parameter("identh", [128, 128], f8, isOutput=False)
    y_d = nc.declare_dram_parameter("ypart", [T, C], f16, isOutput=True)
    a_d = nc.declare_dram_parameter("apart", [T, TE], f16, isOutput=True)

    xta_r = xta_d.rearrange("(kt p) n -> p kt n", p=128)
    eta_r = eta_d.rearrange("(kt p) n -> p kt n", p=128)
    wq_r = wq_d.rearrange("(kt p) n -> p kt n", p=128)
    wk_r = wk_d.rearrange("(kt p) n -> p kt n", p=128)
    wv_r = wv_d.rearrange("(kt p) n -> p kt n", p=128)
    wp_r = wp_d.rearrange("(kt p) n -> p kt n", p=128)
    mneg_r = mneg_d.rearrange("(tc p) s -> p tc s", p=128)

    with tile.TileContext(nc) as tc:
        with tc.tile_pool(name="persist", bufs=1) as persist:
            qT = persist.tile([128, 4, T], f16, tag="qT")
            kT = persist.tile([128, 4, TE], f16, tag="kT")
            vsb = persist.tile([128, 8, 512], f16, tag="vsb")
            wp = persist.tile([128, 4, C], f16, tag="wp")
            mka = persist.tile([128, 16, TE], f8, tag="mka")
            idh = persist.tile([128, 128], f8, tag="idh")
            bqk = persist.tile([128, 8], f32, tag="bqk")
            eb = persist.tile([128, 1], f32, tag="eb")

            # ---------------- stage A: projections (K, V, then Q) ----------
            # loads are chunked and ordered so the first matmul can start
            # after ~2MB (wk + half of enc^T) instead of the full ~12MB
            psA_ctx = tc.tile_pool(name="psA", bufs=2, space="PSUM")
            psA = psA_ctx.__enter__()
            with (
                tc.tile_pool(name="wkvpool", bufs=1) as wkvpool,
                tc.tile_pool(name="inpool", bufs=1) as inpool,
            ):
                wk = wkvpool.tile([128, KT, 512], f16, tag="wk")
                wv = wkvpool.tile([128, KTV, 512], f16, tag="wv")
                wq = wkvpool.tile([128, KT, 512], f16, tag="wq")
                et = inpool.tile([128, KTV, TE], f16, tag="et")
                xt = inpool.tile([128, KT, T], f16, tag="xt")
                dmy = inpool.tile([128, 16], f16, tag="dmy")

                # DMA triggers fire the moment the issuing engine reaches
                # them, and in-flight transfers round-robin the SDMA rings.
                # So only the immediately-needed loads (wk + first half of
                # enc^T) go out at t=0; later loads are placed behind early
                # K-projection copies in the in-order scalar/gpsimd streams
                # so the first transfers get the full HBM bandwidth.
                nc.scalar.dma_start(out=idh, in_=idh_d[:, :])
                nc.scalar.dma_start(out=bqk, in_=bqk_d[:, :])
                nc.vector.memset(eb, EXPB)
                nc.scalar.dma_start(out=wk, in_=wk_r[:, :, :])
                nc.sync.dma_start(out=et[:, :, 0:512], in_=eta_r[:, :, 0:512])

                for sh in range(2):
                    ssl = slice(sh * 512, (sh + 1) * 512)
                    for pt in range(4):
                        ps = psA.tile([128, 512], f32, tag="psA")
                        for kt in range(KT):
                            nc.tensor.matmul(
                                ps[:, :],
                                wk[:, kt, pt * 128:(pt + 1) * 128],
                                et[:, kt, ssl],
                                start=(kt == 0), stop=(kt == KT - 1),
                            )
                        nc.scalar.activation(
                            kT[:, pt, ssl], ps[:, :],
                            mybir.ActivationFunctionType.Identity,
                            bias=bqk[:, 4 + pt:5 + pt],
                        )
                        if sh == 0 and pt == 0:
                            nc.scalar.dma_start(out=wv, in_=wv_r[:, :, :])
                            nc.scalar.dma_start(
                                out=et[:, :, 512:1024],
                                in_=eta_r[:, :, 512:1024])
                            nc.gpsimd.tensor_copy(dmy, kT[:, 0, 0:16])
                            nc.gpsimd.dma_start(out=mka, in_=mneg_r[:, :, :])
                        if sh == 0 and pt == 2:
                            nc.scalar.dma_start(out=wq, in_=wq_r[:, :, :])
                            nc.scalar.dma_start(
                                out=xt[:, :, 0:1024], in_=xta_r[:, :, 0:1024])
                    for st4 in range(4):
                        st = sh * 4 + st4
                        ps = psA.tile([128, 512], f32, tag="psA")
                        for kt in range(KTV):
                            nc.tensor.matmul(
                                ps[:, :],
                                et[:, kt, st * 128:(st + 1) * 128],
                                wv[:, kt, :],
                                start=(kt == 0), stop=(kt == KTV - 1),
                            )
                        nc.scalar.copy(vsb[:, st, :], ps[:, :])
                        if sh == 0 and st4 == 0:
                            nc.scalar.dma_start(
                                out=xt[:, :, 1024:2048],
                                in_=xta_r[:, :, 1024:2048])
                            nc.scalar.dma_start(out=wp, in_=wp_r[:, :, :])
                for Tc in range(4):
                    tsl = slice(Tc * 512, (Tc + 1) * 512)
                    for pt in range(4):
                        ps = psA.tile([128, 512], f32, tag="psA")
                        for kt in range(KT):
                            nc.tensor.matmul(
                                ps[:, :],
                                wq[:, kt, pt * 128:(pt + 1) * 128],
                                xt[:, kt, tsl],
                                start=(kt == 0), stop=(kt == KT - 1),
                            )
                        nc.scalar.activation(
                            qT[:, pt, tsl], ps[:, :],
                            mybir.ActivationFunctionType.Identity,
                            bias=bqk[:, pt:pt + 1],
                        )
            psA_ctx.__exit__(None, None, None)

            # ---------------- stage B: attention ----------------
            with (
                tc.tile_pool(name="spool", bufs=3, space="PSUM") as spool,
                tc.tile_pool(name="ypool", bufs=1, space="PSUM") as ypool,
                tc.tile_pool(name="ppool", bufs=1, space="PSUM") as ppool,
                tc.tile_pool(name="attEpool", bufs=2) as attEpool,
                tc.tile_pool(name="tmppool", bufs=2) as tmppool,
                tc.tile_pool(name="attTpool", bufs=3) as attTpool,
                tc.tile_pool(name="zpoolZ", bufs=8) as zpoolZ,
                tc.tile_pool(name="zpoolR", bufs=8) as zpoolR,
                tc.tile_pool(name="treepool", bufs=2) as treepool,
                tc.tile_pool(name="accpool", bufs=2) as accpool,
                tc.tile_pool(name="ytpool", bufs=2) as ytpool,
                tc.tile_pool(name="opool", bufs=2) as opool,
            ):
                def scores_pair(tci, p, aE, tmp):
                    """QK+mask matmuls, exp, 1/Z and prob-normalize for heads
                    (2p, 2p+1) of chunk tci."""
                    tsl = slice(tci * 128, (tci + 1) * 128)
                    S0 = spool.tile([128, TE], f32, tag="S")
                    S1 = spool.tile([128, TE], f32, tag="S")
                    for sh in range(2):
                        ssl = slice(sh * 512, (sh + 1) * 512)
                        for h2, S in ((0, S0), (1, S1)):
                            hrow = slice(h2 * 64, (h2 + 1) * 64)
                            nc.tensor.matmul(
                                S[:, ssl],
                                qT[hrow, p, tsl],
                                kT[hrow, p, ssl],
                                start=True, stop=False,
                                tile_position=(h2 * 64, 0),
                            )
                    for h2, S in ((0, S0), (1, S1)):
                        for sh in range(2):
                            ssl = slice(sh * 512, (sh + 1) * 512)
                            nc.tensor.matmul(
                                S[:, ssl], idh[:, :], mka[:, tci, ssl],
                                start=False, stop=True,
                            )
                    Zp = zpoolZ.tile([128, 2], f32, tag="Zp")
                    for h2, S in ((0, S0), (1, S1)):
                        nc.scalar.activation(
                            aE[:, p * 2 + h2, :], S[:, :],
                            mybir.ActivationFunctionType.Exp,
                            bias=eb[:, 0:1],
                            accum_out=Zp[:, h2:h2 + 1],
                        )
                    rcp = zpoolR.tile([128, 2], f32, tag="rcp")
                    nc.vector.reciprocal(rcp[:, :], Zp[:, :])
                    for h2 in range(2):
                        h = p * 2 + h2
                        nc.vector.tensor_scalar_mul(
                            tmp[:, h, :], aE[:, h, :], rcp[:, h2:h2 + 1])

                def av_pair(p, aT, yps4):
                    """AV matmuls for heads (2p, 2p+1) into yps4[:, p, :]."""
                    for st in range(8):
                        for h2 in range(2):
                            h = p * 2 + h2
                            nc.tensor.matmul(
                                yps4[h2 * 64:(h2 + 1) * 64, p, :],
                                vsb[:, st, h * 64:(h + 1) * 64],
                                aT[:, h * 8 + st, :],
                                start=(st == 0), stop=(st == 7),
                                tile_position=(0, h2 * 64),
                            )

                def yts_cast(yps4):
                    """PSUM -> SBUF f16 cast of the AV result."""
                    yts = ytpool.tile([128, 4, 128], f16, tag="yts")
                    nc.vector.tensor_copy(yts[:, :, :], yps4[:, :, :])
                    return yts

                def proj_mm(tci, yts):
                    """Output projection + DMA for chunk tci."""
                    tsl = slice(tci * 128, (tci + 1) * 128)
                    oc = opool.tile([128, C], f16, tag="oc")
                    for ch in range(2):
                        csl = slice(ch * 512, (ch + 1) * 512)
                        pps = ppool.tile([128, 512], f32, tag="pps")
                        for kt in range(4):
                            nc.tensor.matmul(
                                pps[:, :], yts[:, kt, :], wp[:, kt, csl],
                                start=(kt == 0), stop=(kt == 3),
                            )
                        nc.vector.tensor_copy(oc[:, csl], pps[:, :])
                    nc.gpsimd.dma_start(out=y_d[tsl, :], in_=oc[:, :])

                def att_tree(tmp, tree):
                    """First tree levels for att-mean: DVE sums heads 0-3 at
                    2x f16; gpsimd sums heads 4-7."""
                    t01, t23, tA, t45, t67, tB = tree
                    nc.vector.tensor_add(t01, tmp[:, 0, :], tmp[:, 1, :])
                    nc.vector.tensor_add(t23, tmp[:, 2, :], tmp[:, 3, :])
                    nc.vector.tensor_add(tA, t01, t23)
                    nc.gpsimd.tensor_add(t45, tmp[:, 4, :], tmp[:, 5, :])
                    nc.gpsimd.tensor_add(t67, tmp[:, 6, :], tmp[:, 7, :])
                    nc.gpsimd.tensor_add(tB, t45, t67)

                def att_final(tci, tree):
                    tsl = slice(tci * 128, (tci + 1) * 128)
                    acc = accpool.tile([128, TE], f16, tag="acc")
                    nc.vector.tensor_add(acc[:, :], tree[2], tree[5])
                    nc.gpsimd.dma_start(out=a_d[tsl, :], in_=acc[:, :])

                def tree_tiles():
                    return [treepool.tile([128, TE], f16, tag=t, name=t)
                            for t in ("t01", "t23", "tA", "t45", "t67", "tB")]

                # software pipeline, all relative to the score/exp stream of
                # chunk i: att-mean tree of i-1, AV of i-2 (interleaved with
                # the score matmuls), output projection of i-3 (emitted right
                # after the first score pair so the PE reaches chunk i+1's
                # scores without a long boundary stall).
                p1 = None   # (tci, aT, tmp, tree) one chunk behind
                p2 = None   # two chunks behind
                pyts = None  # (tci, yts) projection input, three behind
                for tci in range(16):
                    aE = attEpool.tile([128, HG, TE], f16, tag="aE")
                    tmp = tmppool.tile([128, HG, TE], f16, tag="tmp")
                    aT = attTpool.tile([128, HG * 8, 128], f16, tag="aT")
                    if p2 is not None:
                        yps4 = ypool.tile([128, 4, 128], f32, tag="yps4")
                    for p in range(4):
                        scores_pair(tci, p, aE, tmp)
                        if p == 0:
                            if pyts is not None:
                                proj_mm(*pyts)
                                pyts = None
                            if p1 is not None:
                                att_tree(p1[2], p1[3])
                        if p == 1:
                            nc.sync.dma_start_transpose(
                                aT[:, 0:32, :], tmp[:, 0:4, :])
                        if p2 is not None:
                            av_pair(p, p2[1], yps4)
                        if p == 3 and p1 is not None:
                            att_final(p1[0], p1[3])
                    nc.sync.dma_start_transpose(aT[:, 32:64, :], tmp[:, 4:8, :])
                    tree = tree_tiles()
                    if p2 is not None:
                        pyts = (p2[0], yts_cast(yps4))
                    p2 = p1
                    p1 = (tci, aT, tmp, tree)
                # flush the last chunks (p1=15, p2=14, pyts=13)
                proj_mm(*pyts)
                att_tree(p1[2], p1[3])
                yps4 = ypool.tile([128, 4, 128], f32, tag="yps4")
                for p in range(4):
                    av_pair(p, p2[1], yps4)
                att_final(p1[0], p1[3])
                proj_mm(p2[0], yts_cast(yps4))
                yps4b = ypool.tile([128, 4, 128], f32, tag="yps4")
                for p in range(4):
                    av_pair(p, p1[1], yps4b)
                proj_mm(p1[0], yts_cast(yps4b))

    _split_waits(nc)
    return nc


_PROGRAM = None


def _get_program():
    global _PROGRAM
    if _PROGRAM is None:
        _PROGRAM = _build_program()
    return _PROGRAM


def _host_inputs(x, encoder_output, mask, Wq, bq, Wk, bk, Wv, bv, Wp, bp):
    """Build the 8 per-core input maps."""
    x = np.asarray(x, np.float32)
    enc = np.asarray(encoder_output, np.float32)
    mask = np.asarray(mask)
    scale = 1.0 / np.sqrt(D)
    f8np = ml_dtypes.float8_e4m3
    identh = np.eye(128, dtype=f8np)

    in_maps = []
    for c in range(N_CORES):
        b, hg = c // 2, c % 2
        hsl = slice(hg * 512, (hg + 1) * 512)

        xta = np.ascontiguousarray(x[b].T, dtype=np.float16)
        eta = np.zeros((CV_PAD, TE), np.float16)
        eta[:C] = enc[b].T
        eta[C] = 1.0

        wqta = np.ascontiguousarray(
            (np.asarray(Wq, np.float32)[hsl] * scale).T, dtype=np.float16)
        wkta = np.ascontiguousarray(
            np.asarray(Wk, np.float32)[hsl].T, dtype=np.float16)
        wvta = np.zeros((CV_PAD, 512), np.float16)
        wvta[:C] = np.asarray(Wv, np.float32)[hsl].T
        wvta[C] = np.asarray(bv, np.float32)[hsl]
        wpt = np.ascontiguousarray(
            np.asarray(Wp, np.float32)[:, hsl].T, dtype=np.float16)

        bqk = np.empty((128, 8), np.float32)
        bqk[:, 0:4] = (np.asarray(bq, np.float32)[hsl] * scale).reshape(4, 128).T
        bqk[:, 4:8] = np.asarray(bk, np.float32)[hsl].reshape(4, 128).T

        mneg = np.where(mask[b], np.float32(NEG), np.float32(0)).astype(f8np)

        in_maps.append({
            "xta": xta,
            "eta": eta,
            "wqta": wqta,
            "wkta": wkta,
            "wvta": wvta,
            "wpt": wpt,
            "bqk": bqk,
            "mneg": mneg,
            "identh": identh,
        })
    return in_maps


def kernel(x, encoder_output, mask, Wq, bq, Wk, bk, Wv, bv, Wp, bp):
    nc = _get_program()
    in_maps = _host_inputs(x, encoder_output, mask, Wq, bq, Wk, bk, Wv, bv, Wp, bp)
    trace = bool(int(os.environ.get("KERNEL_TRACE", "0")))
    res = bu.run_bass_kernel_spmd(nc, in_maps, list(range(N_CORES)), trace=trace)
    if trace:
        kernel.last_exec_time_ns = res.exec_time_ns
        kernel.last_profile = res
    outs = res.results

    bp = np.asarray(bp, np.float32)
    y = np.empty((B, T, C), np.float32)
    am = np.empty((B, T, TE), np.float32)
    for b in range(B):
        y[b] = (outs[2 * b]["ypart"].astype(np.float32)
                + outs[2 * b + 1]["ypart"].astype(np.float32) + bp)
        am[b] = (outs[2 * b]["apart"].astype(np.float32)
                 + outs[2 * b + 1]["apart"].astype(np.float32)) * (1.0 / H)
    return (y, am)


# revision 10
# speedup vs baseline: 1.4377x; 1.0125x over previous
"""Cross-attention kernel for Trainium2 (8 NeuronCores, Bass/Tile).

Sharding: core c handles batch b = c//2 and head-group hg = c%2 (8 of 16
heads).  Per (b, hg): k/v/q projections in f16; per-head masked softmax
(scores in PSUM f32, mask folded in via an fp8 identity-matmul
accumulation, exp + row-sum on ACT, probs normalized to f16 on DVE);
att-mean partial as a pairwise tree sum (DVE low half @2x f16, gpsimd high
half); AV on the transposed normalized probs (two xbar DMAs per chunk on
sync) + output projection -> host adds head-group halves + bias, and
scales att-mean by 1/16.

Pipelining: the score/exp stream for chunk i runs concurrently with the
AV/proj of chunk i-2 (two behind, so the PE never takes a long stall that
would re-throttle the HAM clock gate) and the att-mean tree of chunk i-1.
Stage-A loads are chunked and priority-ordered so the first projection
matmul starts ~6us in, with the remaining ~10MB streaming underneath.
"""

import os
import sys

sys.path.insert(0, "/opt/trn_rl_repo")

import numpy as np
import ml_dtypes

import concourse.bass as bass
import concourse.tile as tile
from concourse import mybir
import concourse.bass_utils as bu

# ---------------------------------------------------------------- constants
B, T, TE, C = 4, 2048, 1024, 1024
H = 16          # total heads
HG = 8          # heads per group (per core)
D = 64          # head dim
KT = 8          # contraction tiles for Q/K (bias added in PSUM->SBUF copy)
KTV = 9         # V keeps a ones-row for its bias (v is s-major, bias on free dim)
CV_PAD = KTV * 128
EXPB = -2.0     # constant exp bias (cancels in softmax, guards fp16 overflow)
NEG = -240.0    # additive mask value, exactly representable in fp8 e4m3;
                # exp(s + NEG + EXPB) underflows to exactly 0 for |s| < 200
N_CORES = 8

f32 = mybir.dt.float32
f16 = mybir.dt.float16
f8 = mybir.dt.float8e4


def _split_waits(nc, max_waits=1):
    """walrus in this container accepts at most one sync-wait command per
    instruction; hoist extra waits onto preceding same-engine NoOps."""
    import bass_rust

    ctr = 0
    for f in nc.m.functions:
        for blk in f.blocks:
            il = list(blk.instructions)
            out = []
            changed = False
            for inst in il:
                si = inst.sync_info
                if si is not None and si.on_wait and len(si.on_wait) > max_waits:
                    waits = list(si.on_wait)
                    for w in waits[:-max_waits]:
                        ctr += 1
                        nop = mybir.InstNoOp(name=f"waitsplit_{ctr}", ins=[], outs=[])
                        nop.engine = inst.engine
                        nop.sync_info = bass_rust.SyncInfo(on_wait=[w], on_update=[])
                        out.append(nop)
                    inst.sync_info = bass_rust.SyncInfo(
                        on_wait=waits[-max_waits:],
                        on_update=list(si.on_update) if si.on_update else [],
                    )
                    changed = True
                out.append(inst)
            if changed:
                blk.instructions = out


def _build_program():
    nc = bass.Bass("TRN2", target_bir_lowering=False, debug=False)

    xta_d = nc.declare_dram_parameter("xta", [C, T], f16, isOutput=False)
    eta_d = nc.declare_dram_parameter("eta", [CV_PAD, TE], f16, isOutput=False)
    wq_d = nc.declare_dram_parameter("wqta", [C, 512], f16, isOutput=False)
    wk_d = nc.declare_dram_parameter("wkta", [C, 512], f16, isOutput=False)
    wv_d = nc.declare_dram_parameter("wvta", [CV_PAD, 512], f16, isOutput=False)
    wp_d = nc.declare_dram_parameter("wpt", [512, C], f16, isOutput=False)
    bqk_d = nc.declare_dram_parameter("bqk", [128, 8], f32, isOutput=False)
    mneg_d = nc.declare_dram_parameter("mneg", [T, TE], f8, isOutput=False)
    idh_d = nc.declare_dram_parameter("identh", [128, 128], f8, isOutput=False)
    y_d = nc.declare_dram_parameter("ypart", [T, C], f16, isOutput=True)
    a_d = nc.declare_dram_parameter("apart", [T, TE], f16, isOutput=True)

    xta_r = xta_d.rearrange("(kt p) n -> p kt n", p=128)
    eta_r = eta_d.rearrange("(kt p) n -> p kt n", p=128)
    wq_r = wq_d.rearrange("(kt p) n -> p kt n", p=128)
    wk_r = wk_d.rearrange("(kt p) n -> p kt n", p=128)
    wv_r = wv_d.rearrange("(kt p) n -> p kt n", p=128)
    wp_r = wp_d.rearrange("(kt p) n -> p kt n", p=128)
    mneg_r = mneg_d.rearrange("(tc p) s -> p tc s", p=128)

    with tile.TileContext(nc) as tc:
        with tc.tile_pool(name="persist", bufs=1) as persist:
            qT = persist.tile([128, 4, T], f16, tag="qT")
            kT = persist.tile([128, 4, TE], f16, tag="kT")
            vsb = persist.tile([128, 8, 512], f16, tag="vsb")
            wp = persist.tile([128, 4, C], f16, tag="wp")
            mka = persist.tile([128, 16, TE], f8, tag="mka")
            idh = persist.tile([128, 128], f8, tag="idh")
            bqk = persist.tile([128, 8], f32, tag="bqk")
            eb = persist.tile([128, 1], f32, tag="eb")

            # ---------------- stage A: projections (K, V, then Q) ----------
            # loads are chunked and ordered so the first matmul can start
            # after ~2MB (wk + half of enc^T) instead of the full ~12MB
            psA_ctx = tc.tile_pool(name="psA", bufs=2, space="PSUM")
            psA = psA_ctx.__enter__()
            with (
                tc.tile_pool(name="wkvpool", bufs=1) as wkvpool,
                tc.tile_pool(name="inpool", bufs=1) as inpool,
            ):
                wk = wkvpool.tile([128, KT, 512], f16, tag="wk")
                wv = wkvpool.tile([128, KTV, 512], f16, tag="wv")
                wq = wkvpool.tile([128, KT, 512], f16, tag="wq")
                et = inpool.tile([128, KTV, TE], f16, tag="et")
                xt = inpool.tile([128, KT, T], f16, tag="xt")
                dmy = inpool.tile([128, 16], f16, tag="dmy")

                # DMA triggers fire the moment the issuing engine reaches
                # them, and in-flight transfers round-robin the SDMA rings.
                # So only the immediately-needed loads (wk + first half of
                # enc^T) go out at t=0; later loads are placed behind early
                # K-projection copies in the in-order scalar/gpsimd streams
                # so the first transfers get the full HBM bandwidth.
                nc.scalar.dma_start(out=idh, in_=idh_d[:, :])
                nc.scalar.dma_start(out=bqk, in_=bqk_d[:, :])
                nc.vector.memset(eb, EXPB)
                nc.scalar.dma_start(out=wk, in_=wk_r[:, :, :])
                nc.sync.dma_start(out=et[:, :, 0:512], in_=eta_r[:, :, 0:512])

                for sh in range(2):
                    ssl = slice(sh * 512, (sh + 1) * 512)
                    for pt in range(4):
                        ps = psA.tile([128, 512], f32, tag="psA")
                        for kt in range(KT):
                            nc.tensor.matmul(
                                ps[:, :],
                                wk[:, kt, pt * 128:(pt + 1) * 128],
                                et[:, kt, ssl],
                                start=(kt == 0), stop=(kt == KT - 1),
                            )
                        nc.scalar.activation(
                            kT[:, pt, ssl], ps[:, :],
                            mybir.ActivationFunctionType.Identity,
                            bias=bqk[:, 4 + pt:5 + pt],
                        )
                        if sh == 0 and pt == 0:
                            nc.scalar.dma_start(out=wv, in_=wv_r[:, :, :])
                            nc.scalar.dma_start(
                                out=et[:, :, 512:1024],
                                in_=eta_r[:, :, 512:1024])
                            nc.gpsimd.tensor_copy(dmy, kT[:, 0, 0:16])
                            nc.gpsimd.dma_start(out=mka, in_=mneg_r[:, :, :])
                        if sh == 0 and pt == 2:
                            nc.scalar.dma_start(out=wq, in_=wq_r[:, :, :])
                            nc.scalar.dma_start(
                                out=xt[:, :, 0:1024], in_=xta_r[:, :, 0:1024])
                    for st4 in range(4):
                        st = sh * 4 + st4
                        ps = psA.tile([128, 512], f32, tag="psA")
                        for kt in range(KTV):
                            nc.tensor.matmul(
                                ps[:, :],
                                et[:, kt, st * 128:(st + 1) * 128],
                                wv[:, kt, :],
                                start=(kt == 0), stop=(kt == KTV - 1),
                            )
                        nc.scalar.copy(vsb[:, st, :], ps[:, :])
                        if sh == 0 and st4 == 0:
                            nc.scalar.dma_start(
                                out=xt[:, :, 1024:2048],
                                in_=xta_r[:, :, 1024:2048])
                            nc.scalar.dma_start(out=wp, in_=wp_r[:, :, :])
                for Tc in range(4):
                    tsl = slice(Tc * 512, (Tc + 1) * 512)
                    for pt in range(4):
                        ps = psA.tile([128, 512], f32, tag="psA")
                        for kt in range(KT):
                            nc.tensor.matmul(
                                ps[:, :],
                                wq[:, kt, pt * 128:(pt + 1) * 128],
                                xt[:, kt, tsl],
                                start=(kt == 0), stop=(kt == KT - 1),
                            )
                        nc.scalar.activation(
                            qT[:, pt, tsl], ps[:, :],
                            mybir.ActivationFunctionType.Identity,
                            bias=bqk[:, pt:pt + 1],
                        )
            psA_ctx.__exit__(None, None, None)

            # ---------------- stage B: attention ----------------
            with (
                tc.tile_pool(name="spool", bufs=3, space="PSUM") as spool,
                tc.tile_pool(name="ypool", bufs=1, space="PSUM") as ypool,
                tc.tile_pool(name="ppool", bufs=1, space="PSUM") as ppool,
                tc.tile_pool(name="attEpool", bufs=2) as attEpool,
                tc.tile_pool(name="tmppool", bufs=2) as tmppool,
                tc.tile_pool(name="attTpool", bufs=3) as attTpool,
                tc.tile_pool(name="zpoolZ", bufs=8) as zpoolZ,
                tc.tile_pool(name="zpoolR", bufs=8) as zpoolR,
                tc.tile_pool(name="treepool", bufs=2) as treepool,
                tc.tile_pool(name="accpool", bufs=2) as accpool,
                tc.tile_pool(name="ytpool", bufs=2) as ytpool,
                tc.tile_pool(name="opool", bufs=2) as opool,
            ):
                def scores_pair(tci, p, aE, tmp):
                    """QK+mask matmuls, exp, 1/Z and prob-normalize for heads
                    (2p, 2p+1) of chunk tci."""
                    tsl = slice(tci * 128, (tci + 1) * 128)
                    S0 = spool.tile([128, TE], f32, tag="S")
                    S1 = spool.tile([128, TE], f32, tag="S")
                    for sh in range(2):
                        ssl = slice(sh * 512, (sh + 1) * 512)
                        for h2, S in ((0, S0), (1, S1)):
                            hrow = slice(h2 * 64, (h2 + 1) * 64)
                            nc.tensor.matmul(
                                S[:, ssl],
                                qT[hrow, p, tsl],
                                kT[hrow, p, ssl],
                                start=True, stop=False,
                                tile_position=(h2 * 64, 0),
                            )
                    for h2, S in ((0, S0), (1, S1)):
                        for sh in range(2):
                            ssl = slice(sh * 512, (sh + 1) * 512)
                            nc.tensor.matmul(
                                S[:, ssl], idh[:, :], mka[:, tci, ssl],
                                start=False, stop=True,
                            )
                    Zp = zpoolZ.tile([128, 2], f32, tag="Zp")
                    for h2, S in ((0, S0), (1, S1)):
                        nc.scalar.activation(
                            aE[:, p * 2 + h2, :], S[:, :],
                            mybir.ActivationFunctionType.Exp,
                            bias=eb[:, 0:1],
                            accum_out=Zp[:, h2:h2 + 1],
                        )
                    rcp = zpoolR.tile([128, 2], f32, tag="rcp")
                    nc.vector.reciprocal(rcp[:, :], Zp[:, :])
                    for h2 in range(2):
                        h = p * 2 + h2
                        nc.vector.tensor_scalar_mul(
                            tmp[:, h, :], aE[:, h, :], rcp[:, h2:h2 + 1])

                def av_pair(p, aT, yps4):
                    """AV matmuls for heads (2p, 2p+1) into yps4[:, p, :]."""
                    for st in range(8):
                        for h2 in range(2):
                            h = p * 2 + h2
                            nc.tensor.matmul(
                                yps4[h2 * 64:(h2 + 1) * 64, p, :],
                                vsb[:, st, h * 64:(h + 1) * 64],
                                aT[:, h * 8 + st, :],
                                start=(st == 0), stop=(st == 7),
                                tile_position=(0, h2 * 64),
                            )

                def yts_cast(yps4):
                    """PSUM -> SBUF f16 cast of the AV result."""
                    yts = ytpool.tile([128, 4, 128], f16, tag="yts")
                    nc.vector.tensor_copy(yts[:, :, :], yps4[:, :, :])
                    return yts

                def proj_mm(tci, yts):
                    """Output projection + DMA for chunk tci."""
                    tsl = slice(tci * 128, (tci + 1) * 128)
                    oc = opool.tile([128, C], f16, tag="oc")
                    for ch in range(2):
                        csl = slice(ch * 512, (ch + 1) * 512)
                        pps = ppool.tile([128, 512], f32, tag="pps")
                        for kt in range(4):
                            nc.tensor.matmul(
                                pps[:, :], yts[:, kt, :], wp[:, kt, csl],
                                start=(kt == 0), stop=(kt == 3),
                            )
                        nc.vector.tensor_copy(oc[:, csl], pps[:, :])
                    nc.gpsimd.dma_start(out=y_d[tsl, :], in_=oc[:, :])

                def att_tree(tmp, tree):
                    """First tree levels for att-mean: DVE sums heads 0-3 at
                    2x f16; gpsimd sums heads 4-7."""
                    t01, t23, tA, t45, t67, tB = tree
                    nc.vector.tensor_add(t01, tmp[:, 0, :], tmp[:, 1, :])
                    nc.vector.tensor_add(t23, tmp[:, 2, :], tmp[:, 3, :])
                    nc.vector.tensor_add(tA, t01, t23)
                    nc.gpsimd.tensor_add(t45, tmp[:, 4, :], tmp[:, 5, :])
                    nc.gpsimd.tensor_add(t67, tmp[:, 6, :], tmp[:, 7, :])
                    nc.gpsimd.tensor_add(tB, t45, t67)

                def att_final(tci, tree):
                    tsl = slice(tci * 128, (tci + 1) * 128)
                    acc = accpool.tile([128, TE], f16, tag="acc")
                    nc.vector.tensor_add(acc[:, :], tree[2], tree[5])
                    nc.gpsimd.dma_start(out=a_d[tsl, :], in_=acc[:, :])

                def tree_tiles():
                    return [treepool.tile([128, TE], f16, tag=t, name=t)
                            for t in ("t01", "t23", "tA", "t45", "t67", "tB")]

                # software pipeline, all relative to the score/exp stream of
                # chunk i: att-mean tree of i-1, AV of i-2 (interleaved with
                # the score matmuls), output projection of i-3 (emitted right
                # after the first score pair so the PE reaches chunk i+1's
                # scores without a long boundary stall).
                p1 = None   # (tci, aT, tmp, tree) one chunk behind
                p2 = None   # two chunks behind
                pyts = None  # (tci, yts) projection input, three behind
                for tci in range(16):
                    aE = attEpool.tile([128, HG, TE], f16, tag="aE")
                    tmp = tmppool.tile([128, HG, TE], f16, tag="tmp")
                    aT = attTpool.tile([128, HG * 8, 128], f16, tag="aT")
                    for p in range(4):
                        scores_pair(tci, p, aE, tmp)
                        if p == 0:
                            if pyts is not None:
                                proj_mm(*pyts)
                                pyts = None
                            if p1 is not None:
                                att_tree(p1[2], p1[3])
                        if p == 1:
                            nc.sync.dma_start_transpose(
                                aT[:, 0:32, :], tmp[:, 0:4, :])
                        if p2 is not None:
                            if p == 0:
                                yps4 = ypool.tile([128, 4, 128], f32,
                                                  tag="yps4")
                            av_pair(p, p2[1], yps4)
                        if p == 3 and p1 is not None:
                            att_final(p1[0], p1[3])
                    nc.sync.dma_start_transpose(aT[:, 32:64, :], tmp[:, 4:8, :])
                    tree = tree_tiles()
                    if p2 is not None:
                        pyts = (p2[0], yts_cast(yps4))
                    p2 = p1
                    p1 = (tci, aT, tmp, tree)
                # flush the last chunks (p1=15, p2=14, pyts=13)
                proj_mm(*pyts)
                att_tree(p1[2], p1[3])
                yps4 = ypool.tile([128, 4, 128], f32, tag="yps4")
                for p in range(4):
                    av_pair(p, p2[1], yps4)
                att_final(p1[0], p1[3])
                proj_mm(p2[0], yts_cast(yps4))
                yps4b = ypool.tile([128, 4, 128], f32, tag="yps4")
                for p in range(4):
                    av_pair(p, p1[1], yps4b)
                proj_mm(p1[0], yts_cast(yps4b))

    _split_waits(nc)
    return nc


_PROGRAM = None


def _get_program():
    global _PROGRAM
    if _PROGRAM is None:
        _PROGRAM = _build_program()
    return _PROGRAM


def _host_inputs(x, encoder_output, mask, Wq, bq, Wk, bk, Wv, bv, Wp, bp):
    """Build the 8 per-core input maps."""
    x = np.asarray(x, np.float32)
    enc = np.asarray(encoder_output, np.float32)
    mask = np.asarray(mask)
    scale = 1.0 / np.sqrt(D)
    f8np = ml_dtypes.float8_e4m3
    identh = np.eye(128, dtype=f8np)

    in_maps = []
    for c in range(N_CORES):
        b, hg = c // 2, c % 2
        hsl = slice(hg * 512, (hg + 1) * 512)

        xta = np.ascontiguousarray(x[b].T, dtype=np.float16)
        eta = np.zeros((CV_PAD, TE), np.float16)
        eta[:C] = enc[b].T
        eta[C] = 1.0

        wqta = np.ascontiguousarray(
            (np.asarray(Wq, np.float32)[hsl] * scale).T, dtype=np.float16)
        wkta = np.ascontiguousarray(
            np.asarray(Wk, np.float32)[hsl].T, dtype=np.float16)
        wvta = np.zeros((CV_PAD, 512), np.float16)
        wvta[:C] = np.asarray(Wv, np.float32)[hsl].T
        wvta[C] = np.asarray(bv, np.float32)[hsl]
        wpt = np.ascontiguousarray(
            np.asarray(Wp, np.float32)[:, hsl].T, dtype=np.float16)

        bqk = np.empty((128, 8), np.float32)
        bqk[:, 0:4] = (np.asarray(bq, np.float32)[hsl] * scale).reshape(4, 128).T
        bqk[:, 4:8] = np.asarray(bk, np.float32)[hsl].reshape(4, 128).T

        mneg = np.where(mask[b], np.float32(NEG), np.float32(0)).astype(f8np)

        in_maps.append({
            "xta": xta,
            "eta": eta,
            "wqta": wqta,
            "wkta": wkta,
            "wvta": wvta,
            "wpt": wpt,
            "bqk": bqk,
            "mneg": mneg,
            "identh": identh,
        })
    return in_maps


def kernel(x, encoder_output, mask, Wq, bq, Wk, bk, Wv, bv, Wp, bp):
    nc = _get_program()
    in_maps = _host_inputs(x, encoder_output, mask, Wq, bq, Wk, bk, Wv, bv, Wp, bp)
    trace = bool(int(os.environ.get("KERNEL_TRACE", "0")))
    res = bu.run_bass_kernel_spmd(nc, in_maps, list(range(N_CORES)), trace=trace)
    if trace:
        kernel.last_exec_time_ns = res.exec_time_ns
        kernel.last_profile = res
    outs = res.results

    bp = np.asarray(bp, np.float32)
    y = np.empty((B, T, C), np.float32)
    am = np.empty((B, T, TE), np.float32)
    for b in range(B):
        y[b] = (outs[2 * b]["ypart"].astype(np.float32)
                + outs[2 * b + 1]["ypart"].astype(np.float32) + bp)
        am[b] = (outs[2 * b]["apart"].astype(np.float32)
                 + outs[2 * b + 1]["apart"].astype(np.float32)) * (1.0 / H)
    return (y, am)
